# revision 17
# baseline (speedup 1.0000x reference)
"""CoPhyNet Trainium2 kernel — 8-core SPMD Bass/Tile implementation.

Self-contained: hardcodes shapes from the problem spec.
  struct_obs_ab: (17, 256, 56) fp32
  struct_obs_c:  (1, 256, 56) fp32

Sharding: the object axis K=256 is split 8 ways (32 "local" objects per
core). All-pairs edge MLPs: pair[p,q] = concat(x[q], x[p]), output index q
(local), mean over p (free axis). Layer 1 is decomposed into an outer sum
U[q] + V[p]; V is streamed over all 256 p as the matmul moving operand,
U enters as the per-partition activation bias. Layers 2/3 run as 4-way
block-diagonal [128,128] @ [128,256] float32r matmuls (4 local objects
packed in the partition dim). The delta loop all-gathers each core's
32-row V contribution (4 KB) per iteration.
"""

import numpy as np

import concourse.bass as bass
import concourse.bacc as bacc
import concourse.tile as tile
import concourse.mybir as mybir
from concourse.bass_utils import run_bass_kernel_spmd

FP = mybir.dt.float32
FR = mybir.dt.float32r
AF = mybir.ActivationFunctionType
ALU = mybir.AluOpType

N_CORES = 8
TAB = 17
K = 256
F = 56
H = 32
BL = K // N_CORES          # local objects per core = 32
NB = BL // 4               # 4-packed blocks per core = 8
TPRED = TAB - 1
G = H + F                  # 88
DEBUG = False


def _np(x):
    return np.asarray(x, dtype=np.float32)


def _blockdiag4(w):
    out = np.zeros((128, 128), dtype=np.float32)
    for j in range(4):
        out[32 * j:32 * j + 32, 32 * j:32 * j + 32] = w
    return out


def _prep_weights(params):
    """Host-side weight preprocessing -> dict of np arrays (DRAM inputs)."""
    d = {}

    def lin(p):
        return _np(p["w"]), _np(p["b"])

    # ---- phase 1: mlp_inter (112->32->32->32) ----
    w1, b1 = lin(params["mlp_inter"][0])
    w2, b2 = lin(params["mlp_inter"][1])
    w3, b3 = lin(params["mlp_inter"][2])
    d["w1t_aug"] = np.concatenate([w1[:F], b1[None, :]], 0)          # (57, 32)
    d["w1b4"] = np.tile(w1[F:], (1, 4))                               # (56, 128)
    d["w2bd"] = _blockdiag4(w2)                                       # (128, 128)
    d["w3bd"] = _blockdiag4(w3)
    d["b2_4"] = np.tile(b2, 4)[:, None]                               # (128, 1)
    d["b3_4"] = np.tile(b3, 4)[:, None]

    # ---- mlp_out (88->32->32), E rows prescaled by 1/K ----
    wo1, bo1 = lin(params["mlp_out"][0])
    wo2, bo2 = lin(params["mlp_out"][1])
    d["wo1a"] = np.concatenate([wo1[:F], bo1[None, :]], 0)            # (57, 32)
    d["wo1b"] = wo1[F:] / K                                           # (32, 32)
    wo2_aug = np.concatenate([wo2, bo2[None, :]], 0)                  # (33, 32)

    # ---- GRUs: split gate weights; x-side fused with the upstream linear
    # (gates = W_ih^T @ (Wup^T @ v) = (Wup @ W_ih)^T @ v, exact) ----
    _gru_raw = {}
    for name, p in [("r", params["rnn"]), ("rd", params["rnn_delta"])]:
        wih, whh = _np(p["w_ih"]), _np(p["w_hh"])
        bih, bhh = _np(p["b_ih"]), _np(p["b_hh"])
        _gru_raw[name] = wih
        for gi, gn in enumerate(("r", "z", "n")):
            d[f"whh_{name}_{gn}"] = whh[:, gi * H:(gi + 1) * H]
        bs = bih + bhh
        d[f"bs_{name}_r"] = bs[0:H, None]                             # (32, 1)
        d[f"bs_{name}_z"] = bs[H:2 * H, None]
        d[f"bhhn_{name}"] = bhh[2 * H:, None]
        d[f"bihn_{name}"] = bih[2 * H:, None]

    # ---- phase 3: mlp_inter_stab (176->32->32->32), xc = [conf, pose] ----
    ws1, bs1 = lin(params["mlp_inter_stab"][0])
    ws2, bs2 = lin(params["mlp_inter_stab"][1])
    ws3, bs3 = lin(params["mlp_inter_stab"][2])
    d["w1st_aug"] = np.concatenate([ws1[:G], bs1[None, :]], 0)        # (89, 32)
    d["w1sb4"] = np.tile(ws1[G:], (1, 4))                             # (88, 128)
    d["w2sbd"] = _blockdiag4(ws2)
    d["w3sbd"] = _blockdiag4(ws3)
    d["b2s_4"] = np.tile(bs2, 4)[:, None]
    d["b3s_4"] = np.tile(bs3, 4)[:, None]

    # ---- mlp_stab (120->32->1), Es rows prescaled ----
    wm1, bm1 = lin(params["mlp_stab"][0])
    wm2, bm2 = lin(params["mlp_stab"][1])
    d["ws1a"] = np.concatenate([wm1[:G], bm1[None, :]], 0)            # (89, 32)
    d["ws1b"] = wm1[G:] / K                                           # (32, 32)
    d["ws2_aug"] = np.concatenate([wm2, bm2[None, :]], 0)             # (33, 1)

    # ---- phase 4: mlp_inter_delta (176->...), xcat = [pose, conf] ----
    wd1, bd1 = lin(params["mlp_inter_delta"][0])
    wd2, bd2 = lin(params["mlp_inter_delta"][1])
    wd3, bd3 = lin(params["mlp_inter_delta"][2])
    d["w1dt_aug"] = np.concatenate([wd1[:G], bd1[None, :]], 0)        # (89, 32)
    d["w1db4"] = np.tile(wd1[G:], (1, 4))                             # (88, 128)
    d["w2dbd"] = _blockdiag4(wd2)
    d["w3dbd"] = _blockdiag4(wd3)
    d["b2d_4"] = np.tile(bd2, 4)[:, None]
    d["b3d_4"] = np.tile(bd3, 4)[:, None]

    # ---- mlp_gcn_delta (120->32->32), Ed rows prescaled ----
    wg1, bg1 = lin(params["mlp_gcn_delta"][0])
    wg2, bg2 = lin(params["mlp_gcn_delta"][1])
    d["wg1a"] = np.concatenate([wg1[:G], bg1[None, :]], 0)            # (89, 32)
    d["wg1b"] = wg1[G:] / K                                           # (32, 32)
    wg2_aug = np.concatenate([wg2, bg2[None, :]], 0)                  # (33, 32)
    for gi, gn in enumerate(("r", "z", "n")):
        d[f"wx_r_{gn}"] = wo2_aug @ _gru_raw["r"][:, gi * H:(gi + 1) * H]
        d[f"wx_rd_{gn}"] = wg2_aug @ _gru_raw["rd"][:, gi * H:(gi + 1) * H]

    # ---- fc_delta (32->56) ----
    wf, bf = lin(params["fc_delta"])
    d["wfc_aug"] = np.concatenate([wf, bf[None, :]], 0)               # (33, 56)

    d["ident56"] = np.eye(F, dtype=np.float32)                        # (56, 56)
    d["ones_1x56"] = np.ones((1, F), dtype=np.float32)                # (1, 56)
    d["ones_fr"] = np.ones((1, K), dtype=np.float32)                  # (1, 256)
    return d


# everything that feeds a matmul is float32r (single-pass PE); fp32 only for
# bias columns (activation bias / tensor_scalar operands)
_FP_WEIGHTS = {
    "b2_4", "b3_4", "b2s_4", "b3s_4", "b2d_4", "b3d_4",
    "bs_r_r", "bs_r_z", "bhhn_r", "bihn_r",
    "bs_rd_r", "bs_rd_z", "bhhn_rd", "bihn_rd",
}


class _P:
    """Pools holder."""


def _interleave(nc, p, psU_ap, n_groups):
    """ub[32j+f, g] = U[f, 4g+j]; psU_ap [32, 4*n_groups] PSUM -> SBUF ub."""
    ub = p.wk.tile([128, n_groups], FP, tag="ub")
    sv = psU_ap.rearrange("f (g j) -> f g j", j=4)
    for j in range(4):
        if j % 2 == 0:
            nc.scalar.copy(ub[32 * j:32 * j + 32, :], sv[:, :, j])
        else:
            nc.vector.tensor_copy(ub[32 * j:32 * j + 32, :], sv[:, :, j])
    return ub


def _deinterleave(nc, dst_ap, src):
    """dst[f, 4g+j] = src[32j+f, g]; dst AP [32, 32] SBUF, src [128, 8]."""
    dv = dst_ap.rearrange("f (g j) -> f g j", j=4)
    for j in range(4):
        if j % 2 == 0:
            nc.scalar.copy(dv[:, :, j], src[32 * j:32 * j + 32, :])
        else:
            nc.vector.tensor_copy(dv[:, :, j], src[32 * j:32 * j + 32, :])


def _edge_blocks(nc, p, v4_ap, ub_cols, w2bd, w3bd, b2col, b3col, msum):
    """8 blocks of the 4-packed edge MLP; msum [128, 8] gets per-block sums.

    ub_cols(g) -> [128, 1] bias AP for block g. Elementwise passes alternate
    between ACT and DVE per block parity to balance the two engines.
    """
    for g in range(NB):
        h1 = p.blk.tile([128, 256], FR, tag="h1")
        if g % 2 == 0:
            nc.scalar.activation(h1[:], v4_ap, AF.Relu, bias=ub_cols(g))
        else:
            nc.vector.scalar_tensor_tensor(h1[:], v4_ap, ub_cols(g),
                                           p.zeros[:], op0=ALU.add,
                                           op1=ALU.max)
        ps2 = p.ps2.tile([128, 256], FP, tag="mm")
        nc.tensor.matmul(ps2[:], w2bd[:], h1[:], start=True, stop=True)
        h2 = p.blk.tile([128, 256], FR, tag="h2")
        if g % 2 == 0:
            nc.vector.tensor_scalar(h2[:], ps2[:], b2col, 0.0,
                                    op0=ALU.add, op1=ALU.max)
        else:
            nc.scalar.activation(h2[:], ps2[:], AF.Relu, bias=b2col)
        ps3 = p.ps3.tile([128, 256], FP, tag="mm")
        nc.tensor.matmul(ps3[:], w3bd[:], h2[:], start=True, stop=True)
        e3 = p.blk.tile([128, 256], FP, tag="e3")
        if g % 2 == 0:
            nc.scalar.activation(e3[:], ps3[:], AF.Relu, bias=b3col,
                                 accum_out=msum[:, g:g + 1])
        else:
            nc.vector.scalar_tensor_tensor(e3[:], ps3[:], b3col, p.zeros[:],
                                           op0=ALU.add, op1=ALU.max,
                                           accum_out=msum[:, g:g + 1])


def _gru_step(nc, p, W, pre, x_ap, h_ap):
    """One feature-major GRU cell step; h_ap [32, BL] updated in place.

    x_ap is the pre-GRU relu vector (with ones row); the upstream linear is
    folded into the wx_* gate weights.
    """
    ps_r = p.pssm.tile([H, BL], FP, tag="sm")
    nc.tensor.matmul(ps_r[:], W[f"wx_{pre}_r"][:], x_ap,
                     start=True, stop=False)
    nc.tensor.matmul(ps_r[:], W[f"whh_{pre}_r"][:], h_ap,
                     start=False, stop=True)
    r = p.wk.tile([H, BL], FP, tag="r")
    nc.scalar.activation(r[:], ps_r[:], AF.Sigmoid, bias=W[f"bs_{pre}_r"][:])
    ps_z = p.pssm.tile([H, BL], FP, tag="sm")
    nc.tensor.matmul(ps_z[:], W[f"wx_{pre}_z"][:], x_ap,
                     start=True, stop=False)
    nc.tensor.matmul(ps_z[:], W[f"whh_{pre}_z"][:], h_ap,
                     start=False, stop=True)
    z = p.wk.tile([H, BL], FP, tag="z")
    nc.scalar.activation(z[:], ps_z[:], AF.Sigmoid, bias=W[f"bs_{pre}_z"][:])
    ps_gin = p.pssm.tile([H, BL], FP, tag="sm")
    nc.tensor.matmul(ps_gin[:], W[f"wx_{pre}_n"][:], x_ap,
                     start=True, stop=True)
    ps_ghn = p.pssm.tile([H, BL], FP, tag="sm")
    nc.tensor.matmul(ps_ghn[:], W[f"whh_{pre}_n"][:], h_ap,
                     start=True, stop=True)
    hn = p.wk.tile([H, BL], FP, tag="hn")
    nc.scalar.activation(hn[:], ps_ghn[:], AF.Identity,
                         bias=W[f"bhhn_{pre}"][:])
    rhn = p.wk.tile([H, BL], FP, tag="rhn")
    nc.vector.tensor_mul(rhn[:], r[:], hn[:])
    npre = p.wk.tile([H, BL], FP, tag="npre")
    nc.vector.tensor_add(npre[:], ps_gin[:], rhn[:])
    nt = p.wk.tile([H, BL], FP, tag="nt")
    nc.scalar.activation(nt[:], npre[:], AF.Tanh, bias=W[f"bihn_{pre}"][:])
    hmn = p.wk.tile([H, BL], FP, tag="hmn")
    nc.vector.tensor_sub(hmn[:], h_ap, nt[:])
    zh = p.wk.tile([H, BL], FP, tag="zh")
    nc.vector.tensor_mul(zh[:], z[:], hmn[:])
    nc.vector.tensor_add(h_ap, nt[:], zh[:])


def build_program(wshapes):
    """Build + compile the 8-core SPMD program. wshapes: weight name->shape."""
    nc = bacc.Bacc("TRN2", target_bir_lowering=False, debug=False,
                   num_devices=N_CORES)

    # ---------- DRAM I/O ----------
    Wd = {}
    for name, shp in wshapes.items():
        dt = FP if name in _FP_WEIGHTS else FR
        Wd[name] = nc.dram_tensor(name, list(shp), dt,
                                  kind="ExternalInput").ap()

    xfT_d = nc.dram_tensor("xfT", [TAB, F, K], FR, kind="ExternalInput").ap()
    # all 17 t's of local x, feature-major with ones row: (57, 544)
    xla_d = nc.dram_tensor("xlT_all", [F + 1, TAB * BL], FR,
                           kind="ExternalInput").ap()
    pose0T_full_d = nc.dram_tensor("pose0T_full", [F, K], FR,
                                   kind="ExternalInput").ap()
    pose0T_loc_d = nc.dram_tensor("pose0T_loc", [F, BL], FR,
                                  kind="ExternalInput").ap()
    pose0_loc_b_d = nc.dram_tensor("pose0_loc_b", [BL, F], FP,
                                   kind="ExternalInput").ap()

    poses_out = nc.dram_tensor("poses_loc", [TPRED, BL, F], FP,
                               kind="ExternalOutput").ap()
    stab_out = nc.dram_tensor("stab_loc", [1, BL], FP,
                              kind="ExternalOutput").ap()
    dbg = {}
    if DEBUG:
        for nm, shp in [("dbg_conf", [H, BL]), ("dbg_em0", [H, BL]),
                        ("dbg_es", [H, BL]), ("dbg_ed0", [H, BL])]:
            dbg[nm] = nc.dram_tensor(nm, shp, FR, kind="ExternalOutput").ap()
        dbg["dbg_mask"] = nc.dram_tensor("dbg_mask", [F, BL], FP,
                                         kind="ExternalOutput").ap()

    rg = [list(range(N_CORES))]

    with tile.TileContext(nc) as tc:
        with (
            tc.tile_pool(name="const", bufs=1) as cpool,
            tc.tile_pool(name="state", bufs=1) as st,
            tc.tile_pool(name="xin", bufs=4) as xin,
            tc.tile_pool(name="work", bufs=6) as wk,
            tc.tile_pool(name="blk", bufs=5) as blk,
            tc.tile_pool(name="psV", bufs=2, space="PSUM") as psV_pool,
            tc.tile_pool(name="psmm", bufs=4, space="PSUM") as psmm_pool,
            tc.tile_pool(name="pssm", bufs=2, space="PSUM") as pssm,
            tc.tile_pool(name="dram", bufs=2, space="DRAM") as dram,
        ):
            p = _P()
            p.wk, p.blk, p.pssm = wk, blk, pssm
            p.ps2, p.ps3 = psmm_pool, psmm_pool
            p.zeros = cpool.tile([128, 256], FP, tag="zeros")
            nc.vector.memset(p.zeros[:], 0.0)

            # ---- load constants into SBUF ----
            W = {}
            for name, shp in wshapes.items():
                dt = FP if name in _FP_WEIGHTS else FR
                t = cpool.tile(list(shp), dt, tag=f"c_{name}")
                nc.sync.dma_start(t[:], Wd[name][:])
                W[name] = t

            # ---- persistent state ----
            hconf = st.tile([H, BL], FR, tag="hconf")     # phase-2 GRU state
            nc.vector.tensor_copy(hconf[:], p.zeros[0:H, 0:BL])
            # ginT = xcat^T local: rows 0:56 pose, 56:88 conf, 88 ones
            ginT = st.tile([G + 1, BL], FR, tag="ginT")
            nc.sync.dma_start(ginT[0:F, :], pose0T_loc_d[:])
            nc.sync.dma_start(ginT[G:G + 1, :], Wd["ones_fr"][:, 0:BL])
            # xcT_full: rows 0:32 conf^T full, 32:88 pose0^T full, 88 ones
            xcT = st.tile([G + 1, K], FR, tag="xcT")
            nc.sync.dma_start(xcT[H:G, :], pose0T_full_d[:])
            nc.sync.dma_start(xcT[G:G + 1, :], Wd["ones_fr"][:])
            # xcl = xc^T local: rows 0:32 conf, 32:88 pose, 88 ones
            xcl = st.tile([G + 1, BL], FR, tag="xcl")
            nc.sync.dma_start(xcl[H:G, :], pose0T_loc_d[:])
            nc.sync.dma_start(xcl[G:G + 1, :], Wd["ones_fr"][:, 0:BL])
            # hd_aug: GRU-delta state + ones row
            hd_aug = st.tile([H + 1, BL], FR, tag="hd_aug")
            nc.vector.tensor_copy(hd_aug[0:H, :], p.zeros[0:H, 0:BL])
            nc.sync.dma_start(hd_aug[H:H + 1, :], Wd["ones_fr"][:, 0:BL])
            pose_b = st.tile([BL, F], FP, tag="pose_b")   # b-major pose copy
            nc.sync.dma_start(pose_b[:], pose0_loc_b_d[:])
            mask56 = st.tile([F, BL], FR, tag="mask56")
            # persistent relu tiles with ones rows
            q1 = st.tile([H + 1, BL], FR, tag="q1")
            nc.sync.dma_start(q1[H:H + 1, :], Wd["ones_fr"][:, 0:BL])
            s1 = st.tile([H + 1, BL], FR, tag="s1")
            nc.sync.dma_start(s1[H:H + 1, :], Wd["ones_fr"][:, 0:BL])
            g1 = st.tile([H + 1, BL], FR, tag="g1")
            nc.sync.dma_start(g1[H:H + 1, :], Wd["ones_fr"][:, 0:BL])
            edT = st.tile([H, BL], FR, tag="edT")         # delta-loop E means
            # full xcat^T = [pose; conf] over all 256 objects (phase-4 V side)
            xdT = st.tile([G, K], FR, tag="xdT")
            nc.sync.dma_start(xdT[0:F, :], pose0T_full_d[:])
            # whole local x batch (feature-major + ones rows)
            xla = st.tile([F + 1, TAB * BL], FR, tag="xla")
            nc.sync.dma_start(xla[:], xla_d[:])

            # ============ batched U for phase 1: ub_all [128, 136] ==========
            # U cols are (t, b): col = 32t + b, b = 4g + j; ub col = 8t + g
            ub_all = st.tile([128, TAB * NB], FP, tag="ub_all")
            for c0, c1 in [(0, 256), (256, TAB * BL)]:
                psUh = pssm.tile([H, c1 - c0], FP, tag="sm")
                nc.tensor.matmul(psUh[:], W["w1t_aug"][:], xla[:, c0:c1],
                                 start=True, stop=True)
                sv = psUh[:, :].rearrange("f (g j) -> f g j", j=4)
                gc0 = c0 // 4
                ng = (c1 - c0) // 4
                for j in range(4):
                    if j % 2 == 0:
                        nc.scalar.copy(
                            ub_all[32 * j:32 * j + 32, gc0:gc0 + ng],
                            sv[:, :, j])
                    else:
                        nc.vector.tensor_copy(
                            ub_all[32 * j:32 * j + 32, gc0:gc0 + ng],
                            sv[:, :, j])

            # ================= phase 1 + 2: gcn_on_AB + GRU =================
            for t in range(TAB):
                xf = xin.tile([F, K], FR, tag="xf")
                nc.sync.dma_start(xf[:], xfT_d[t])

                psV = psV_pool.tile([128, K], FP, tag="psV")
                nc.tensor.matmul(psV[:], W["w1b4"][:], xf[:],
                                 start=True, stop=True)
                msum = wk.tile([128, NB], FP, tag="msum")
                _edge_blocks(nc, p, psV[:],
                             lambda g, t=t: ub_all[:, 8 * t + g:8 * t + g + 1],
                             W["w2bd"], W["w3bd"],
                             W["b2_4"][:], W["b3_4"][:], msum)

                emT = wk.tile([H, BL], FR, tag="emT")
                _deinterleave(nc, emT[:, :], msum)

                pso1 = pssm.tile([H, BL], FP, tag="sm")
                nc.tensor.matmul(pso1[:], W["wo1a"][:],
                                 xla[:, t * BL:(t + 1) * BL],
                                 start=True, stop=False)
                nc.tensor.matmul(pso1[:], W["wo1b"][:], emT[:],
                                 start=False, stop=True)
                nc.scalar.activation(q1[0:H, :], pso1[:], AF.Relu)
                if DEBUG and t == 0:
                    nc.sync.dma_start(dbg["dbg_em0"][:], emT[:])

                _gru_step(nc, p, W, "r", q1[:], hconf[:])

            # conf into ginT/xcl (SBUF->SBUF DMA handles row offsets)
            nc.sync.dma_start(ginT[F:G, :], hconf[:])
            nc.sync.dma_start(xcl[0:H, :], hconf[:])
            if DEBUG:
                nc.sync.dma_start(dbg["dbg_conf"][:], hconf[:])

            # ================= conf AllGather =================
            cin = dram.tile([H, BL], FR, tag="cin")
            nc.sync.dma_start(cin[:], hconf[:])
            cout = dram.tile([K, BL], FR, tag="cout")
            nc.gpsimd.collective_compute(
                "AllGather", ALU.bypass, replica_groups=rg,
                ins=[cin.opt()], outs=[cout.opt()])
            cview = cout[:, :].rearrange("(r f) b -> f r b", f=H)
            nc.sync.dma_start(
                xcT[0:H, :].rearrange("f (r b) -> f r b", b=BL), cview)
            nc.sync.dma_start(
                xdT[F:G, :].rearrange("f (r b) -> f r b", b=BL), cview)

            # ================= phase 3: pred_stab =================
            psUs = pssm.tile([H, BL], FP, tag="sm")
            nc.tensor.matmul(psUs[:], W["w1st_aug"][:], xcl[:],
                             start=True, stop=True)
            ubs = _interleave(nc, p, psUs[:, :], NB)
            psVs = psV_pool.tile([128, K], FP, tag="psV")
            nc.tensor.matmul(psVs[:], W["w1sb4"][:], xcT[0:G, :],
                             start=True, stop=True)
            msums = wk.tile([128, NB], FP, tag="msum")
            _edge_blocks(nc, p, psVs[:], lambda g: ubs[:, g:g + 1],
                         W["w2sbd"], W["w3sbd"],
                         W["b2s_4"][:], W["b3s_4"][:], msums)
            esT = wk.tile([H, BL], FR, tag="esT")
            _deinterleave(nc, esT[:, :], msums)
            if DEBUG:
                nc.sync.dma_start(dbg["dbg_es"][:], esT[:])

            pss1 = pssm.tile([H, BL], FP, tag="sm")
            nc.tensor.matmul(pss1[:], W["ws1a"][:], xcl[:],
                             start=True, stop=False)
            nc.tensor.matmul(pss1[:], W["ws1b"][:], esT[:],
                             start=False, stop=True)
            nc.scalar.activation(s1[0:H, :], pss1[:], AF.Relu)
            pss2 = pssm.tile([1, BL], FP, tag="sm")
            nc.tensor.matmul(pss2[:], W["ws2_aug"][:], s1[:],
                             start=True, stop=True)
            stab_sb = wk.tile([1, BL], FP, tag="stab_sb")
            nc.scalar.copy(stab_sb[:], pss2[:])
            nc.sync.dma_start(stab_out[:], stab_sb[:])
            # mask row: 1.0 where stab <= 0
            maskr = wk.tile([1, BL], FR, tag="maskr")
            nc.vector.tensor_scalar(maskr[:], pss2[:], 0.0, None,
                                    op0=ALU.is_le)
            psm = pssm.tile([F, BL], FP, tag="sm")
            nc.tensor.matmul(psm[:], W["ones_1x56"][:], maskr[:],
                             start=True, stop=True)
            nc.vector.tensor_copy(mask56[:], psm[:])
            if DEBUG:
                mask56fp = wk.tile([F, BL], FP, tag="mask56fp")
                nc.vector.tensor_copy(mask56fp[:], psm[:])
                nc.sync.dma_start(dbg["dbg_mask"][:], mask56fp[:])

            # ================= phase 4: delta loop =================
            for i in range(TPRED):
                psV4 = psV_pool.tile([128, K], FP, tag="psV")
                nc.tensor.matmul(psV4[:], W["w1db4"][:], xdT[:],
                                 start=True, stop=True)
                psUd = pssm.tile([H, BL], FP, tag="sm")
                nc.tensor.matmul(psUd[:], W["w1dt_aug"][:], ginT[:],
                                 start=True, stop=True)
                ubd = _interleave(nc, p, psUd[:, :], NB)
                msumd = wk.tile([128, NB], FP, tag="msum")
                _edge_blocks(nc, p, psV4[:], lambda g: ubd[:, g:g + 1],
                             W["w2dbd"], W["w3dbd"],
                             W["b2d_4"][:], W["b3d_4"][:], msumd)
                _deinterleave(nc, edT[:, :], msumd)
                if DEBUG and i == 0:
                    nc.sync.dma_start(dbg["dbg_ed0"][:], edT[:])

                psg1 = pssm.tile([H, BL], FP, tag="sm")
                nc.tensor.matmul(psg1[:], W["wg1a"][:], ginT[:],
                                 start=True, stop=False)
                nc.tensor.matmul(psg1[:], W["wg1b"][:], edT[:],
                                 start=False, stop=True)
                nc.scalar.activation(g1[0:H, :], psg1[:], AF.Relu)

                _gru_step(nc, p, W, "rd", g1[:], hd_aug[0:H, :])

                psd = pssm.tile([F, BL], FP, tag="sm")
                nc.tensor.matmul(psd[:], W["wfc_aug"][:], hd_aug[:],
                                 start=True, stop=True)
                delta = wk.tile([F, BL], FR, tag="delta")
                nc.vector.tensor_mul(delta[:], psd[:], mask56[:])
                # pose update (feature-major, in place)
                nc.vector.tensor_add(ginT[0:F, :], ginT[0:F, :], delta[:])

                # b-major pose snapshot -> DRAM output
                psdT = pssm.tile([BL, F], FR, tag="sm")
                nc.tensor.transpose(psdT[:], delta[:], W["ident56"][:])
                nc.vector.tensor_add(pose_b[:], pose_b[:], psdT[:])
                nc.sync.dma_start(poses_out[i], pose_b[:])

                if i < TPRED - 1:
                    # all-gather this iteration's delta; update full xcat
                    din = dram.tile([F, BL], FR, tag="din")
                    nc.sync.dma_start(din[:], delta[:])
                    dout = dram.tile([N_CORES * F, BL], FR, tag="dout")
                    nc.gpsimd.collective_compute(
                        "AllGather", ALU.bypass, replica_groups=rg,
                        ins=[din.opt()], outs=[dout.opt()])
                    dfull = wk.tile([F, K], FR, tag="dfull")
                    dv = dout[:, :].rearrange("(r f) b -> f r b", f=F)
                    nc.sync.dma_start(
                        dfull[:, :].rearrange("f (r b) -> f r b", b=BL), dv)
                    nc.vector.tensor_add(xdT[0:F, :], xdT[0:F, :], dfull[:])

    nc.compile()
    return nc


_CACHE = {}


def kernel(struct_obs_ab, struct_obs_c, params):
    x_ab = _np(struct_obs_ab)            # (17, 256, 56)
    pose0 = _np(struct_obs_c)[0]         # (256, 56)

    wd = _prep_weights(params)
    wshapes = {k: v.shape for k, v in wd.items()}

    if "prog" not in _CACHE:
        _CACHE["prog"] = build_program(wshapes)
    nc = _CACHE["prog"]

    xfT = np.ascontiguousarray(x_ab.transpose(0, 2, 1))   # (17, 56, 256)
    pose0T = np.ascontiguousarray(pose0.T)                # (56, 256)

    in_maps = []
    for c in range(N_CORES):
        sl = slice(c * BL, (c + 1) * BL)
        # (57, 17*32): col 32t+b = [x_ab[t, local b]; 1]
        xla = np.concatenate(
            [x_ab[:, sl, :].transpose(0, 2, 1),
             np.ones((TAB, 1, BL), np.float32)], axis=1)   # (17, 57, 32)
        xla = np.ascontiguousarray(
            xla.transpose(1, 0, 2).reshape(F + 1, TAB * BL))
        m = dict(wd)
        m["xfT"] = xfT
        m["xlT_all"] = xla
        m["pose0T_full"] = pose0T
        m["pose0T_loc"] = np.ascontiguousarray(pose0T[:, sl])
        m["pose0_loc_b"] = np.ascontiguousarray(pose0[sl, :])
        in_maps.append(m)

    res = run_bass_kernel_spmd(nc, in_maps, core_ids=list(range(N_CORES)))
    _CACHE["last_results"] = res

    poses = np.zeros((1, TPRED, K, F), np.float32)
    stab = np.zeros((1, K), np.float32)
    for c in range(N_CORES):
        sl = slice(c * BL, (c + 1) * BL)
        poses[0, :, sl, :] = res.results[c]["poses_loc"]
        stab[0, sl] = res.results[c]["stab_loc"][0]

    stability = np.broadcast_to(stab[:, None, :], (1, TPRED, K)).copy()
    return poses, stability


# revision 18
# speedup vs baseline: 1.0115x; 1.0115x over previous
"""CoPhyNet Trainium2 kernel — 8-core SPMD Bass/Tile implementation.

Self-contained: hardcodes shapes from the problem spec.
  struct_obs_ab: (17, 256, 56) fp32
  struct_obs_c:  (1, 256, 56) fp32

Sharding: the object axis K=256 is split 8 ways (32 "local" objects per
core). All-pairs edge MLPs: pair[p,q] = concat(x[q], x[p]), output index q
(local), mean over p (free axis). Layer 1 is decomposed into an outer sum
U[q] + V[p]; V is streamed over all 256 p as the matmul moving operand,
U enters as the per-partition activation bias. Layers 2/3 run as 4-way
block-diagonal [128,128] @ [128,256] float32r matmuls (4 local objects
packed in the partition dim). The delta loop all-gathers each core's
32-row V contribution (4 KB) per iteration.
"""

import numpy as np

import concourse.bass as bass
import concourse.bacc as bacc
import concourse.tile as tile
import concourse.mybir as mybir
from concourse.bass_utils import run_bass_kernel_spmd

FP = mybir.dt.float32
FR = mybir.dt.float32r
AF = mybir.ActivationFunctionType
ALU = mybir.AluOpType

N_CORES = 8
TAB = 17
K = 256
F = 56
H = 32
BL = K // N_CORES          # local objects per core = 32
NB = BL // 4               # 4-packed blocks per core = 8
TPRED = TAB - 1
G = H + F                  # 88
DEBUG = False


def _np(x):
    return np.asarray(x, dtype=np.float32)


def _blockdiag4(w):
    out = np.zeros((128, 128), dtype=np.float32)
    for j in range(4):
        out[32 * j:32 * j + 32, 32 * j:32 * j + 32] = w
    return out


def _prep_weights(params):
    """Host-side weight preprocessing -> dict of np arrays (DRAM inputs)."""
    d = {}

    def lin(p):
        return _np(p["w"]), _np(p["b"])

    # ---- phase 1: mlp_inter (112->32->32->32) ----
    w1, b1 = lin(params["mlp_inter"][0])
    w2, b2 = lin(params["mlp_inter"][1])
    w3, b3 = lin(params["mlp_inter"][2])
    d["w1t_aug"] = np.concatenate([w1[:F], b1[None, :]], 0)          # (57, 32)
    d["w1b4"] = np.tile(w1[F:], (1, 4))                               # (56, 128)
    d["w2bd"] = _blockdiag4(w2)                                       # (128, 128)
    d["w3bd"] = _blockdiag4(w3)
    d["b2_4"] = np.tile(b2, 4)[:, None]                               # (128, 1)
    d["b3_4"] = np.tile(b3, 4)[:, None]

    # ---- mlp_out (88->32->32), E rows prescaled by 1/K ----
    wo1, bo1 = lin(params["mlp_out"][0])
    wo2, bo2 = lin(params["mlp_out"][1])
    d["wo1a"] = np.concatenate([wo1[:F], bo1[None, :]], 0)            # (57, 32)
    d["wo1b"] = wo1[F:] / K                                           # (32, 32)


    # ---- GRUs: split gate weights; x-side fused with the upstream linear
    # (gates = W_ih^T @ (Wup^T @ v) = (Wup @ W_ih)^T @ v, exact) ----
    _gru_raw = {}
    for name, p in [("r", params["rnn"]), ("rd", params["rnn_delta"])]:
        wih, whh = _np(p["w_ih"]), _np(p["w_hh"])
        bih, bhh = _np(p["b_ih"]), _np(p["b_hh"])
        _gru_raw[name] = wih
        for gi, gn in enumerate(("r", "z", "n")):
            d[f"whh_{name}_{gn}"] = whh[:, gi * H:(gi + 1) * H]
        bs = bih + bhh
        d[f"bs_{name}_r"] = bs[0:H, None]                             # (32, 1)
        d[f"bs_{name}_z"] = bs[H:2 * H, None]
        d[f"bhhn_{name}"] = bhh[2 * H:, None]
        d[f"bihn_{name}"] = bih[2 * H:, None]

    # ---- phase 3: mlp_inter_stab (176->32->32->32), xc = [conf, pose] ----
    ws1, bs1 = lin(params["mlp_inter_stab"][0])
    ws2, bs2 = lin(params["mlp_inter_stab"][1])
    ws3, bs3 = lin(params["mlp_inter_stab"][2])
    d["w1st_aug"] = np.concatenate([ws1[:G], bs1[None, :]], 0)        # (89, 32)
    d["w1sb4"] = np.tile(ws1[G:], (1, 4))                             # (88, 128)
    d["w2sbd"] = _blockdiag4(ws2)
    d["w3sbd"] = _blockdiag4(ws3)
    d["b2s_4"] = np.tile(bs2, 4)[:, None]
    d["b3s_4"] = np.tile(bs3, 4)[:, None]

    # ---- mlp_stab (120->32->1), Es rows prescaled ----
    wm1, bm1 = lin(params["mlp_stab"][0])
    wm2, bm2 = lin(params["mlp_stab"][1])
    d["ws1a"] = np.concatenate([wm1[:G], bm1[None, :]], 0)            # (89, 32)
    d["ws1b"] = wm1[G:] / K                                           # (32, 32)
    d["ws2_aug"] = np.concatenate([wm2, bm2[None, :]], 0)             # (33, 1)

    # ---- phase 4: mlp_inter_delta (176->...), xcat = [pose, conf] ----
    wd1, bd1 = lin(params["mlp_inter_delta"][0])
    wd2, bd2 = lin(params["mlp_inter_delta"][1])
    wd3, bd3 = lin(params["mlp_inter_delta"][2])
    d["w1dt_aug"] = np.concatenate([wd1[:G], bd1[None, :]], 0)        # (89, 32)
    d["w1db4"] = np.tile(wd1[G:], (1, 4))                             # (88, 128)
    d["w2dbd"] = _blockdiag4(wd2)
    d["w3dbd"] = _blockdiag4(wd3)
    d["b2d_4"] = np.tile(bd2, 4)[:, None]
    d["b3d_4"] = np.tile(bd3, 4)[:, None]

    # ---- mlp_gcn_delta (120->32->32), Ed rows prescaled ----
    wg1, bg1 = lin(params["mlp_gcn_delta"][0])
    wg2, bg2 = lin(params["mlp_gcn_delta"][1])
    d["wg1a"] = np.concatenate([wg1[:G], bg1[None, :]], 0)            # (89, 32)
    d["wg1b"] = wg1[G:] / K                                           # (32, 32)
    for gi, gn in enumerate(("r", "z", "n")):
        d[f"wx_r_{gn}"] = wo2 @ _gru_raw["r"][:, gi * H:(gi + 1) * H]
        d[f"wx_rd_{gn}"] = wg2 @ _gru_raw["rd"][:, gi * H:(gi + 1) * H]
    # fold the upstream linear's bias through the gate weights
    for nm, bias in [("r", bo2), ("rd", bg2)]:
        wih = _gru_raw[nm]
        d[f"bs_{nm}_r"] = d[f"bs_{nm}_r"] + (bias @ wih[:, 0:H])[:, None]
        d[f"bs_{nm}_z"] = d[f"bs_{nm}_z"] + (bias @ wih[:, H:2 * H])[:, None]
        d[f"bihn_{nm}"] = d[f"bihn_{nm}"] + (bias @ wih[:, 2 * H:])[:, None]

    # ---- fc_delta (32->56) ----
    wf, bf = lin(params["fc_delta"])
    d["wfc_aug"] = np.concatenate([wf, bf[None, :]], 0)               # (33, 56)

    d["ident56"] = np.eye(F, dtype=np.float32)                        # (56, 56)
    d["ones_1x56"] = np.ones((1, F), dtype=np.float32)                # (1, 56)
    d["ones_fr"] = np.ones((1, K), dtype=np.float32)                  # (1, 256)
    return d


# everything that feeds a matmul is float32r (single-pass PE); fp32 only for
# bias columns (activation bias / tensor_scalar operands)
_FP_WEIGHTS = {
    "b2_4", "b3_4", "b2s_4", "b3s_4", "b2d_4", "b3d_4",
    "bs_r_r", "bs_r_z", "bhhn_r", "bihn_r",
    "bs_rd_r", "bs_rd_z", "bhhn_rd", "bihn_rd",
}


class _P:
    """Pools holder."""


def _interleave(nc, p, psU_ap, n_groups):
    """ub[32j+f, g] = U[f, 4g+j]; psU_ap [32, 4*n_groups] PSUM -> SBUF ub."""
    ub = p.wk.tile([128, n_groups], FP, tag="ub")
    sv = psU_ap.rearrange("f (g j) -> f g j", j=4)
    for j in range(4):
        if j % 2 == 0:
            nc.scalar.copy(ub[32 * j:32 * j + 32, :], sv[:, :, j])
        else:
            nc.vector.tensor_copy(ub[32 * j:32 * j + 32, :], sv[:, :, j])
    return ub


def _deinterleave(nc, dst_ap, src):
    """dst[f, 4g+j] = src[32j+f, g]; dst AP [32, 32] SBUF, src [128, 8]."""
    dv = dst_ap.rearrange("f (g j) -> f g j", j=4)
    for j in range(4):
        if j % 2 == 0:
            nc.scalar.copy(dv[:, :, j], src[32 * j:32 * j + 32, :])
        else:
            nc.vector.tensor_copy(dv[:, :, j], src[32 * j:32 * j + 32, :])


def _edge_blocks(nc, p, v4_ap, ub_cols, w2bd, w3bd, b2col, b3col, msum):
    """8 blocks of the 4-packed edge MLP; msum [128, 8] gets per-block sums.

    ub_cols(g) -> [128, 1] bias AP for block g. Elementwise passes alternate
    between ACT and DVE per block parity to balance the two engines.
    """
    for g in range(NB):
        h1 = p.blk.tile([128, 256], FR, tag="h1")
        if g % 2 == 0:
            nc.scalar.activation(h1[:], v4_ap, AF.Relu, bias=ub_cols(g))
        else:
            nc.vector.scalar_tensor_tensor(h1[:], v4_ap, ub_cols(g),
                                           p.zeros[:], op0=ALU.add,
                                           op1=ALU.max)
        ps2 = p.ps2.tile([128, 256], FP, tag="mm")
        nc.tensor.matmul(ps2[:], w2bd[:], h1[:], start=True, stop=True)
        h2 = p.blk.tile([128, 256], FR, tag="h2")
        if g % 2 == 0:
            nc.vector.tensor_scalar(h2[:], ps2[:], b2col, 0.0,
                                    op0=ALU.add, op1=ALU.max)
        else:
            nc.scalar.activation(h2[:], ps2[:], AF.Relu, bias=b2col)
        ps3 = p.ps3.tile([128, 256], FP, tag="mm")
        nc.tensor.matmul(ps3[:], w3bd[:], h2[:], start=True, stop=True)
        e3 = p.blk.tile([128, 256], FP, tag="e3")
        if g % 2 == 0:
            nc.scalar.activation(e3[:], ps3[:], AF.Relu, bias=b3col,
                                 accum_out=msum[:, g:g + 1])
        else:
            nc.vector.scalar_tensor_tensor(e3[:], ps3[:], b3col, p.zeros[:],
                                           op0=ALU.add, op1=ALU.max,
                                           accum_out=msum[:, g:g + 1])


def _gru_step(nc, p, W, pre, x_ap, h_ap):
    """One feature-major GRU cell step; h_ap [32, BL] updated in place.

    x_ap is the pre-GRU relu vector (with ones row); the upstream linear is
    folded into the wx_* gate weights.
    """
    ps_r = p.pssm.tile([H, BL], FP, tag="sm")
    nc.tensor.matmul(ps_r[:], W[f"wx_{pre}_r"][:], x_ap,
                     start=True, stop=False)
    nc.tensor.matmul(ps_r[:], W[f"whh_{pre}_r"][:], h_ap,
                     start=False, stop=True)
    r = p.wk.tile([H, BL], FP, tag="r")
    nc.scalar.activation(r[:], ps_r[:], AF.Sigmoid, bias=W[f"bs_{pre}_r"][:])
    ps_z = p.pssm.tile([H, BL], FP, tag="sm")
    nc.tensor.matmul(ps_z[:], W[f"wx_{pre}_z"][:], x_ap,
                     start=True, stop=False)
    nc.tensor.matmul(ps_z[:], W[f"whh_{pre}_z"][:], h_ap,
                     start=False, stop=True)
    z = p.wk.tile([H, BL], FP, tag="z")
    nc.scalar.activation(z[:], ps_z[:], AF.Sigmoid, bias=W[f"bs_{pre}_z"][:])
    ps_gin = p.pssm.tile([H, BL], FP, tag="sm")
    nc.tensor.matmul(ps_gin[:], W[f"wx_{pre}_n"][:], x_ap,
                     start=True, stop=True)
    ps_ghn = p.pssm.tile([H, BL], FP, tag="sm")
    nc.tensor.matmul(ps_ghn[:], W[f"whh_{pre}_n"][:], h_ap,
                     start=True, stop=True)
    hn = p.wk.tile([H, BL], FP, tag="hn")
    nc.scalar.activation(hn[:], ps_ghn[:], AF.Identity,
                         bias=W[f"bhhn_{pre}"][:])
    rhn = p.wk.tile([H, BL], FP, tag="rhn")
    nc.vector.tensor_mul(rhn[:], r[:], hn[:])
    npre = p.wk.tile([H, BL], FP, tag="npre")
    nc.vector.tensor_add(npre[:], ps_gin[:], rhn[:])
    nt = p.wk.tile([H, BL], FP, tag="nt")
    nc.scalar.activation(nt[:], npre[:], AF.Tanh, bias=W[f"bihn_{pre}"][:])
    hmn = p.wk.tile([H, BL], FP, tag="hmn")
    nc.vector.tensor_sub(hmn[:], h_ap, nt[:])
    zh = p.wk.tile([H, BL], FP, tag="zh")
    nc.vector.tensor_mul(zh[:], z[:], hmn[:])
    nc.vector.tensor_add(h_ap, nt[:], zh[:])


def build_program(wshapes):
    """Build + compile the 8-core SPMD program. wshapes: weight name->shape."""
    nc = bacc.Bacc("TRN2", target_bir_lowering=False, debug=False,
                   num_devices=N_CORES)

    # ---------- DRAM I/O ----------
    Wd = {}
    for name, shp in wshapes.items():
        dt = FP if name in _FP_WEIGHTS else FR
        Wd[name] = nc.dram_tensor(name, list(shp), dt,
                                  kind="ExternalInput").ap()

    xfT_d = nc.dram_tensor("xfT", [TAB, F, K], FR, kind="ExternalInput").ap()
    # all 17 t's of local x, feature-major with ones row: (57, 544)
    xla_d = nc.dram_tensor("xlT_all", [F + 1, TAB * BL], FR,
                           kind="ExternalInput").ap()
    pose0T_full_d = nc.dram_tensor("pose0T_full", [F, K], FR,
                                   kind="ExternalInput").ap()
    pose0T_loc_d = nc.dram_tensor("pose0T_loc", [F, BL], FR,
                                  kind="ExternalInput").ap()
    pose0_loc_b_d = nc.dram_tensor("pose0_loc_b", [BL, F], FP,
                                   kind="ExternalInput").ap()

    poses_out = nc.dram_tensor("poses_loc", [TPRED, BL, F], FP,
                               kind="ExternalOutput").ap()
    stab_out = nc.dram_tensor("stab_loc", [1, BL], FP,
                              kind="ExternalOutput").ap()
    dbg = {}
    if DEBUG:
        for nm, shp in [("dbg_conf", [H, BL]), ("dbg_em0", [H, BL]),
                        ("dbg_es", [H, BL]), ("dbg_ed0", [H, BL])]:
            dbg[nm] = nc.dram_tensor(nm, shp, FR, kind="ExternalOutput").ap()
        dbg["dbg_mask"] = nc.dram_tensor("dbg_mask", [F, BL], FP,
                                         kind="ExternalOutput").ap()

    rg = [list(range(N_CORES))]

    with tile.TileContext(nc) as tc:
        with (
            tc.tile_pool(name="const", bufs=1) as cpool,
            tc.tile_pool(name="state", bufs=1) as st,
            tc.tile_pool(name="xin", bufs=4) as xin,
            tc.tile_pool(name="work", bufs=6) as wk,
            tc.tile_pool(name="blk", bufs=5) as blk,
            tc.tile_pool(name="psV", bufs=2, space="PSUM") as psV_pool,
            tc.tile_pool(name="psmm", bufs=4, space="PSUM") as psmm_pool,
            tc.tile_pool(name="pssm", bufs=2, space="PSUM") as pssm,
            tc.tile_pool(name="dram", bufs=2, space="DRAM") as dram,
        ):
            p = _P()
            p.wk, p.blk, p.pssm = wk, blk, pssm
            p.ps2, p.ps3 = psmm_pool, psmm_pool
            p.zeros = cpool.tile([128, 256], FP, tag="zeros")
            nc.vector.memset(p.zeros[:], 0.0)

            # ---- load constants into SBUF ----
            W = {}
            for name, shp in wshapes.items():
                dt = FP if name in _FP_WEIGHTS else FR
                t = cpool.tile(list(shp), dt, tag=f"c_{name}")
                nc.sync.dma_start(t[:], Wd[name][:])
                W[name] = t

            # ---- persistent state ----
            hconf = st.tile([H, BL], FR, tag="hconf")     # phase-2 GRU state
            nc.vector.tensor_copy(hconf[:], p.zeros[0:H, 0:BL])
            # ginT = xcat^T local: rows 0:56 pose, 56:88 conf, 88 ones
            ginT = st.tile([G + 1, BL], FR, tag="ginT")
            nc.sync.dma_start(ginT[0:F, :], pose0T_loc_d[:])
            nc.sync.dma_start(ginT[G:G + 1, :], Wd["ones_fr"][:, 0:BL])
            # xcT_full: rows 0:32 conf^T full, 32:88 pose0^T full, 88 ones
            xcT = st.tile([G + 1, K], FR, tag="xcT")
            nc.sync.dma_start(xcT[H:G, :], pose0T_full_d[:])
            nc.sync.dma_start(xcT[G:G + 1, :], Wd["ones_fr"][:])
            # xcl = xc^T local: rows 0:32 conf, 32:88 pose, 88 ones
            xcl = st.tile([G + 1, BL], FR, tag="xcl")
            nc.sync.dma_start(xcl[H:G, :], pose0T_loc_d[:])
            nc.sync.dma_start(xcl[G:G + 1, :], Wd["ones_fr"][:, 0:BL])
            # hd_aug: GRU-delta state + ones row
            hd_aug = st.tile([H + 1, BL], FR, tag="hd_aug")
            nc.vector.tensor_copy(hd_aug[0:H, :], p.zeros[0:H, 0:BL])
            nc.sync.dma_start(hd_aug[H:H + 1, :], Wd["ones_fr"][:, 0:BL])
            pose_b = st.tile([BL, F], FP, tag="pose_b")   # b-major pose copy
            nc.sync.dma_start(pose_b[:], pose0_loc_b_d[:])
            mask56 = st.tile([F, BL], FR, tag="mask56")
            # persistent relu tile with ones row (stab head)
            s1 = st.tile([H + 1, BL], FR, tag="s1")
            nc.sync.dma_start(s1[H:H + 1, :], Wd["ones_fr"][:, 0:BL])
            # full xcat^T = [pose; conf] over all 256 objects (phase-4 V side)
            xdT = st.tile([G, K], FR, tag="xdT")
            nc.sync.dma_start(xdT[0:F, :], pose0T_full_d[:])
            # whole local x batch (feature-major + ones rows)
            xla = st.tile([F + 1, TAB * BL], FR, tag="xla")
            nc.sync.dma_start(xla[:], xla_d[:])

            # ============ batched U for phase 1: ub_all [128, 136] ==========
            # U cols are (t, b): col = 32t + b, b = 4g + j; ub col = 8t + g
            ub_all = st.tile([128, TAB * NB], FP, tag="ub_all")
            for c0, c1 in [(0, 256), (256, TAB * BL)]:
                psUh = pssm.tile([H, c1 - c0], FP, tag="sm")
                nc.tensor.matmul(psUh[:], W["w1t_aug"][:], xla[:, c0:c1],
                                 start=True, stop=True)
                sv = psUh[:, :].rearrange("f (g j) -> f g j", j=4)
                gc0 = c0 // 4
                ng = (c1 - c0) // 4
                for j in range(4):
                    if j % 2 == 0:
                        nc.scalar.copy(
                            ub_all[32 * j:32 * j + 32, gc0:gc0 + ng],
                            sv[:, :, j])
                    else:
                        nc.vector.tensor_copy(
                            ub_all[32 * j:32 * j + 32, gc0:gc0 + ng],
                            sv[:, :, j])

            # ================= phase 1 + 2: gcn_on_AB + GRU =================
            for t in range(TAB):
                xf = xin.tile([F, K], FR, tag="xf")
                nc.sync.dma_start(xf[:], xfT_d[t])

                psV = psV_pool.tile([128, K], FP, tag="psV")
                nc.tensor.matmul(psV[:], W["w1b4"][:], xf[:],
                                 start=True, stop=True)
                msum = wk.tile([128, NB], FP, tag="msum")
                _edge_blocks(nc, p, psV[:],
                             lambda g, t=t: ub_all[:, 8 * t + g:8 * t + g + 1],
                             W["w2bd"], W["w3bd"],
                             W["b2_4"][:], W["b3_4"][:], msum)

                emT = wk.tile([H, BL], FR, tag="emT")
                _deinterleave(nc, emT[:, :], msum)

                pso1 = pssm.tile([H, BL], FP, tag="sm")
                nc.tensor.matmul(pso1[:], W["wo1a"][:],
                                 xla[:, t * BL:(t + 1) * BL],
                                 start=True, stop=False)
                nc.tensor.matmul(pso1[:], W["wo1b"][:], emT[:],
                                 start=False, stop=True)
                q1 = wk.tile([H, BL], FR, tag="q1")
                nc.scalar.activation(q1[:], pso1[:], AF.Relu)
                if DEBUG and t == 0:
                    nc.sync.dma_start(dbg["dbg_em0"][:], emT[:])

                _gru_step(nc, p, W, "r", q1[:], hconf[:])

            # conf into ginT/xcl (SBUF->SBUF DMA handles row offsets)
            nc.sync.dma_start(ginT[F:G, :], hconf[:])
            nc.sync.dma_start(xcl[0:H, :], hconf[:])
            if DEBUG:
                nc.sync.dma_start(dbg["dbg_conf"][:], hconf[:])

            # ================= conf AllGather =================
            cin = dram.tile([H, BL], FR, tag="cin")
            nc.sync.dma_start(cin[:], hconf[:])
            cout = dram.tile([K, BL], FR, tag="cout")
            nc.gpsimd.collective_compute(
                "AllGather", ALU.bypass, replica_groups=rg,
                ins=[cin.opt()], outs=[cout.opt()])
            cview = cout[:, :].rearrange("(r f) b -> f r b", f=H)
            nc.sync.dma_start(
                xcT[0:H, :].rearrange("f (r b) -> f r b", b=BL), cview)
            nc.sync.dma_start(
                xdT[F:G, :].rearrange("f (r b) -> f r b", b=BL), cview)

            # ================= phase 3: pred_stab =================
            psUs = pssm.tile([H, BL], FP, tag="sm")
            nc.tensor.matmul(psUs[:], W["w1st_aug"][:], xcl[:],
                             start=True, stop=True)
            ubs = _interleave(nc, p, psUs[:, :], NB)
            psVs = psV_pool.tile([128, K], FP, tag="psV")
            nc.tensor.matmul(psVs[:], W["w1sb4"][:], xcT[0:G, :],
                             start=True, stop=True)
            msums = wk.tile([128, NB], FP, tag="msum")
            _edge_blocks(nc, p, psVs[:], lambda g: ubs[:, g:g + 1],
                         W["w2sbd"], W["w3sbd"],
                         W["b2s_4"][:], W["b3s_4"][:], msums)
            esT = wk.tile([H, BL], FR, tag="esT")
            _deinterleave(nc, esT[:, :], msums)
            if DEBUG:
                nc.sync.dma_start(dbg["dbg_es"][:], esT[:])

            pss1 = pssm.tile([H, BL], FP, tag="sm")
            nc.tensor.matmul(pss1[:], W["ws1a"][:], xcl[:],
                             start=True, stop=False)
            nc.tensor.matmul(pss1[:], W["ws1b"][:], esT[:],
                             start=False, stop=True)
            nc.scalar.activation(s1[0:H, :], pss1[:], AF.Relu)
            pss2 = pssm.tile([1, BL], FP, tag="sm")
            nc.tensor.matmul(pss2[:], W["ws2_aug"][:], s1[:],
                             start=True, stop=True)
            stab_sb = wk.tile([1, BL], FP, tag="stab_sb")
            nc.scalar.copy(stab_sb[:], pss2[:])
            nc.sync.dma_start(stab_out[:], stab_sb[:])
            # mask row: 1.0 where stab <= 0
            maskr = wk.tile([1, BL], FR, tag="maskr")
            nc.vector.tensor_scalar(maskr[:], pss2[:], 0.0, None,
                                    op0=ALU.is_le)
            psm = pssm.tile([F, BL], FP, tag="sm")
            nc.tensor.matmul(psm[:], W["ones_1x56"][:], maskr[:],
                             start=True, stop=True)
            nc.vector.tensor_copy(mask56[:], psm[:])
            if DEBUG:
                mask56fp = wk.tile([F, BL], FP, tag="mask56fp")
                nc.vector.tensor_copy(mask56fp[:], psm[:])
                nc.sync.dma_start(dbg["dbg_mask"][:], mask56fp[:])

            # ================= phase 4: delta loop =================
            for i in range(TPRED):
                psV4 = psV_pool.tile([128, K], FP, tag="psV")
                nc.tensor.matmul(psV4[:], W["w1db4"][:], xdT[:],
                                 start=True, stop=True)
                psUd = pssm.tile([H, BL], FP, tag="sm")
                nc.tensor.matmul(psUd[:], W["w1dt_aug"][:], ginT[:],
                                 start=True, stop=True)
                ubd = _interleave(nc, p, psUd[:, :], NB)
                msumd = wk.tile([128, NB], FP, tag="msum")
                _edge_blocks(nc, p, psV4[:], lambda g: ubd[:, g:g + 1],
                             W["w2dbd"], W["w3dbd"],
                             W["b2d_4"][:], W["b3d_4"][:], msumd)
                edT = wk.tile([H, BL], FR, tag="edT")
                _deinterleave(nc, edT[:, :], msumd)
                if DEBUG and i == 0:
                    nc.sync.dma_start(dbg["dbg_ed0"][:], edT[:])

                psg1 = pssm.tile([H, BL], FP, tag="sm")
                nc.tensor.matmul(psg1[:], W["wg1a"][:], ginT[:],
                                 start=True, stop=False)
                nc.tensor.matmul(psg1[:], W["wg1b"][:], edT[:],
                                 start=False, stop=True)
                g1 = wk.tile([H, BL], FR, tag="g1")
                nc.scalar.activation(g1[:], psg1[:], AF.Relu)

                _gru_step(nc, p, W, "rd", g1[:], hd_aug[0:H, :])

                psd = pssm.tile([F, BL], FP, tag="sm")
                nc.tensor.matmul(psd[:], W["wfc_aug"][:], hd_aug[:],
                                 start=True, stop=True)
                delta = wk.tile([F, BL], FR, tag="delta")
                nc.vector.tensor_mul(delta[:], psd[:], mask56[:])
                # pose update (feature-major, in place)
                nc.vector.tensor_add(ginT[0:F, :], ginT[0:F, :], delta[:])

                # b-major pose snapshot -> DRAM output
                psdT = pssm.tile([BL, F], FR, tag="sm")
                nc.tensor.transpose(psdT[:], delta[:], W["ident56"][:])
                nc.vector.tensor_add(pose_b[:], pose_b[:], psdT[:])
                nc.sync.dma_start(poses_out[i], pose_b[:])

                if i < TPRED - 1:
                    # all-gather this iteration's delta; update full xcat
                    din = dram.tile([F, BL], FR, tag="din")
                    nc.sync.dma_start(din[:], delta[:])
                    dout = dram.tile([N_CORES * F, BL], FR, tag="dout")
                    nc.gpsimd.collective_compute(
                        "AllGather", ALU.bypass, replica_groups=rg,
                        ins=[din.opt()], outs=[dout.opt()])
                    dfull = wk.tile([F, K], FR, tag="dfull")
                    dv = dout[:, :].rearrange("(r f) b -> f r b", f=F)
                    nc.sync.dma_start(
                        dfull[:, :].rearrange("f (r b) -> f r b", b=BL), dv)
                    nc.vector.tensor_add(xdT[0:F, :], xdT[0:F, :], dfull[:])

    nc.compile()
    return nc


_CACHE = {}


def kernel(struct_obs_ab, struct_obs_c, params):
    x_ab = _np(struct_obs_ab)            # (17, 256, 56)
    pose0 = _np(struct_obs_c)[0]         # (256, 56)

    wd = _prep_weights(params)
    wshapes = {k: v.shape for k, v in wd.items()}

    if "prog" not in _CACHE:
        _CACHE["prog"] = build_program(wshapes)
    nc = _CACHE["prog"]

    xfT = np.ascontiguousarray(x_ab.transpose(0, 2, 1))   # (17, 56, 256)
    pose0T = np.ascontiguousarray(pose0.T)                # (56, 256)

    in_maps = []
    for c in range(N_CORES):
        sl = slice(c * BL, (c + 1) * BL)
        # (57, 17*32): col 32t+b = [x_ab[t, local b]; 1]
        xla = np.concatenate(
            [x_ab[:, sl, :].transpose(0, 2, 1),
             np.ones((TAB, 1, BL), np.float32)], axis=1)   # (17, 57, 32)
        xla = np.ascontiguousarray(
            xla.transpose(1, 0, 2).reshape(F + 1, TAB * BL))
        m = dict(wd)
        m["xfT"] = xfT
        m["xlT_all"] = xla
        m["pose0T_full"] = pose0T
        m["pose0T_loc"] = np.ascontiguousarray(pose0T[:, sl])
        m["pose0_loc_b"] = np.ascontiguousarray(pose0[sl, :])
        in_maps.append(m)

    res = run_bass_kernel_spmd(nc, in_maps, core_ids=list(range(N_CORES)))
    _CACHE["last_results"] = res

    poses = np.zeros((1, TPRED, K, F), np.float32)
    stab = np.zeros((1, K), np.float32)
    for c in range(N_CORES):
        sl = slice(c * BL, (c + 1) * BL)
        poses[0, :, sl, :] = res.results[c]["poses_loc"]
        stab[0, sl] = res.results[c]["stab_loc"][0]

    stability = np.broadcast_to(stab[:, None, :], (1, TPRED, K)).copy()
    return poses, stability


# revision 20
# speedup vs baseline: 1.1169x; 1.1041x over previous
"""CoPhyNet Trainium2 kernel — 8-core SPMD Bass/Tile implementation.

Self-contained: hardcodes shapes from the problem spec.
  struct_obs_ab: (17, 256, 56) fp32
  struct_obs_c:  (1, 256, 56) fp32

Sharding: the object axis K=256 is split 8 ways (32 "local" objects per
core). All-pairs edge MLPs: pair[p,q] = concat(x[q], x[p]), output index q
(local), mean over p (free axis). Layer 1 is decomposed into an outer sum
U[q] + V[p]; V is streamed over all 256 p as the matmul moving operand,
U enters as the per-partition activation bias. Layers 2/3 run as 4-way
block-diagonal [128,128] @ [128,256] float32r matmuls (4 local objects
packed in the partition dim). The delta loop all-gathers each core's
32-row V contribution (4 KB) per iteration.
"""

import numpy as np

import concourse.bass as bass
import concourse.bacc as bacc
import concourse.tile as tile
import concourse.mybir as mybir
from concourse.bass_utils import run_bass_kernel_spmd

FP = mybir.dt.float32
FR = mybir.dt.float32r
AF = mybir.ActivationFunctionType
ALU = mybir.AluOpType

N_CORES = 8
TAB = 17
K = 256
F = 56
H = 32
BL = K // N_CORES          # local objects per core = 32
NB = BL // 4               # 4-packed blocks per core = 8
TPRED = TAB - 1
G = H + F                  # 88
DEBUG = False


def _np(x):
    return np.asarray(x, dtype=np.float32)


def _blockdiag4(w):
    out = np.zeros((128, 128), dtype=np.float32)
    for j in range(4):
        out[32 * j:32 * j + 32, 32 * j:32 * j + 32] = w
    return out


def _prep_weights(params):
    """Host-side weight preprocessing -> dict of np arrays (DRAM inputs)."""
    d = {}

    def lin(p):
        return _np(p["w"]), _np(p["b"])

    # ---- phase 1: mlp_inter (112->32->32->32) ----
    w1, b1 = lin(params["mlp_inter"][0])
    w2, b2 = lin(params["mlp_inter"][1])
    w3, b3 = lin(params["mlp_inter"][2])
    d["w1t_aug"] = np.concatenate([w1[:F], b1[None, :]], 0)          # (57, 32)
    d["w1b4"] = np.tile(w1[F:], (1, 4))                               # (56, 128)
    d["w2bd"] = _blockdiag4(w2)                                       # (128, 128)
    d["w3bd"] = _blockdiag4(w3)
    d["b2_4"] = np.tile(b2, 4)[:, None]                               # (128, 1)
    d["b3_4"] = np.tile(b3, 4)[:, None]

    # ---- mlp_out (88->32->32), E rows prescaled by 1/K ----
    wo1, bo1 = lin(params["mlp_out"][0])
    wo2, bo2 = lin(params["mlp_out"][1])
    d["wo1a"] = np.concatenate([wo1[:F], bo1[None, :]], 0)            # (57, 32)
    d["wo1b"] = wo1[F:] / K                                           # (32, 32)


    # ---- GRUs: split gate weights; x-side fused with the upstream linear
    # (gates = W_ih^T @ (Wup^T @ v) = (Wup @ W_ih)^T @ v, exact) ----
    _gru_raw = {}
    for name, p in [("r", params["rnn"]), ("rd", params["rnn_delta"])]:
        wih, whh = _np(p["w_ih"]), _np(p["w_hh"])
        bih, bhh = _np(p["b_ih"]), _np(p["b_hh"])
        _gru_raw[name] = wih
        for gi, gn in enumerate(("r", "z", "n")):
            d[f"whh_{name}_{gn}"] = whh[:, gi * H:(gi + 1) * H]
        bs = bih + bhh
        d[f"bs_{name}_r"] = bs[0:H, None]                             # (32, 1)
        d[f"bs_{name}_z"] = bs[H:2 * H, None]
        d[f"bhhn_{name}"] = bhh[2 * H:, None]
        d[f"bihn_{name}"] = bih[2 * H:, None]

    # ---- phase 3: mlp_inter_stab (176->32->32->32), xc = [conf, pose] ----
    ws1, bs1 = lin(params["mlp_inter_stab"][0])
    ws2, bs2 = lin(params["mlp_inter_stab"][1])
    ws3, bs3 = lin(params["mlp_inter_stab"][2])
    d["w1st_aug"] = np.concatenate([ws1[:G], bs1[None, :]], 0)        # (89, 32)
    d["w1sb4"] = np.tile(ws1[G:], (1, 4))                             # (88, 128)
    d["w2sbd"] = _blockdiag4(ws2)
    d["w3sbd"] = _blockdiag4(ws3)
    d["b2s_4"] = np.tile(bs2, 4)[:, None]
    d["b3s_4"] = np.tile(bs3, 4)[:, None]

    # ---- mlp_stab (120->32->1), Es rows prescaled ----
    wm1, bm1 = lin(params["mlp_stab"][0])
    wm2, bm2 = lin(params["mlp_stab"][1])
    d["ws1a"] = np.concatenate([wm1[:G], bm1[None, :]], 0)            # (89, 32)
    d["ws1b"] = wm1[G:] / K                                           # (32, 32)
    d["ws2_aug"] = np.concatenate([wm2, bm2[None, :]], 0)             # (33, 1)

    # ---- phase 4: mlp_inter_delta (176->...), xcat = [pose, conf] ----
    wd1, bd1 = lin(params["mlp_inter_delta"][0])
    wd2, bd2 = lin(params["mlp_inter_delta"][1])
    wd3, bd3 = lin(params["mlp_inter_delta"][2])
    d["w1dt_aug"] = np.concatenate([wd1[:G], bd1[None, :]], 0)        # (89, 32)
    d["w1db4"] = np.tile(wd1[G:], (1, 4))                             # (88, 128)
    d["w2dbd"] = _blockdiag4(wd2)
    d["w3dbd"] = _blockdiag4(wd3)
    d["b2d_4"] = np.tile(bd2, 4)[:, None]
    d["b3d_4"] = np.tile(bd3, 4)[:, None]

    # ---- mlp_gcn_delta (120->32->32), Ed rows prescaled ----
    wg1, bg1 = lin(params["mlp_gcn_delta"][0])
    wg2, bg2 = lin(params["mlp_gcn_delta"][1])
    d["wg1a"] = np.concatenate([wg1[:G], bg1[None, :]], 0)            # (89, 32)
    d["wg1b"] = wg1[G:] / K                                           # (32, 32)
    for gi, gn in enumerate(("r", "z", "n")):
        d[f"wx_r_{gn}"] = wo2 @ _gru_raw["r"][:, gi * H:(gi + 1) * H]
        d[f"wx_rd_{gn}"] = wg2 @ _gru_raw["rd"][:, gi * H:(gi + 1) * H]
    # fold the upstream linear's bias through the gate weights
    for nm, bias in [("r", bo2), ("rd", bg2)]:
        wih = _gru_raw[nm]
        d[f"bs_{nm}_r"] = d[f"bs_{nm}_r"] + (bias @ wih[:, 0:H])[:, None]
        d[f"bs_{nm}_z"] = d[f"bs_{nm}_z"] + (bias @ wih[:, H:2 * H])[:, None]
        d[f"bihn_{nm}"] = d[f"bihn_{nm}"] + (bias @ wih[:, 2 * H:])[:, None]

    # ---- fc_delta (32->56) ----
    wf, bf = lin(params["fc_delta"])
    d["wfc_aug"] = np.concatenate([wf, bf[None, :]], 0)               # (33, 56)

    d["ident56"] = np.eye(F, dtype=np.float32)                        # (56, 56)
    d["ones_1x56"] = np.ones((1, F), dtype=np.float32)                # (1, 56)
    d["ones_fr"] = np.ones((1, K), dtype=np.float32)                  # (1, 256)
    return d


# everything that feeds a matmul is float32r (single-pass PE); fp32 only for
# bias columns (activation bias / tensor_scalar operands)
_FP_WEIGHTS = {
    "b2_4", "b3_4", "b2s_4", "b3s_4", "b2d_4", "b3d_4",
    "bs_r_r", "bs_r_z", "bhhn_r", "bihn_r",
    "bs_rd_r", "bs_rd_z", "bhhn_rd", "bihn_rd",
}


class _P:
    """Pools holder."""


def _interleave(nc, p, psU_ap, n_groups):
    """ub[32j+f, g] = U[f, 4g+j]; psU_ap [32, 4*n_groups] PSUM -> SBUF ub."""
    ub = p.wk.tile([128, n_groups], FP, tag="ub")
    sv = psU_ap.rearrange("f (g j) -> f g j", j=4)
    for j in range(4):
        if j % 2 == 0:
            nc.scalar.copy(ub[32 * j:32 * j + 32, :], sv[:, :, j])
        else:
            nc.vector.tensor_copy(ub[32 * j:32 * j + 32, :], sv[:, :, j])
    return ub


def _deinterleave(nc, dst_ap, src):
    """dst[f, 4g+j] = src[32j+f, g]; dst AP [32, 32] SBUF, src [128, 8]."""
    dv = dst_ap.rearrange("f (g j) -> f g j", j=4)
    for j in range(4):
        if j % 2 == 0:
            nc.scalar.copy(dv[:, :, j], src[32 * j:32 * j + 32, :])
        else:
            nc.vector.tensor_copy(dv[:, :, j], src[32 * j:32 * j + 32, :])


def _edge_blocks(nc, p, v4_ap, ub_cols, w2bd, w3bd, b2col, b3col, msum):
    """8 blocks of the 4-packed edge MLP, processed as 4 block-PAIRS with
    [128, 512] matmuls/passes; msum [128, 8] gets per-block sums.

    ub_cols(g) -> [128, 1] bias AP for block g.
    """
    for pr in range(NB // 2):
        g0, g1b = 2 * pr, 2 * pr + 1
        h1 = p.blk.tile([128, 512], FR, tag="h1")
        nc.scalar.activation(h1[:, 0:256], v4_ap, AF.Relu, bias=ub_cols(g0))
        nc.vector.scalar_tensor_tensor(h1[:, 256:512], v4_ap, ub_cols(g1b),
                                       p.zeros[:, 0:256],
                                       op0=ALU.add, op1=ALU.max)
        ps2 = p.ps2.tile([128, 512], FP, tag="mm")
        nc.tensor.matmul(ps2[:], w2bd[:], h1[:], start=True, stop=True)
        h2 = p.blk.tile([128, 512], FR, tag="h2")
        if pr % 2 == 0:
            nc.vector.tensor_scalar(h2[:], ps2[:], b2col, 0.0,
                                    op0=ALU.add, op1=ALU.max)
        else:
            nc.scalar.activation(h2[:], ps2[:], AF.Relu, bias=b2col)
        ps3 = p.ps3.tile([128, 512], FP, tag="mm")
        nc.tensor.matmul(ps3[:], w3bd[:], h2[:], start=True, stop=True)
        e3 = p.blk.tile([128, 512], FP, tag="e3")
        if pr % 2 == 0:
            nc.scalar.activation(e3[:], ps3[:], AF.Relu, bias=b3col)
        else:
            nc.vector.scalar_tensor_tensor(e3[:], ps3[:], b3col, p.zeros[:],
                                           op0=ALU.add, op1=ALU.max)
        ev = e3[:, :].rearrange("q (pair a) -> q pair a", pair=2)
        nc.vector.tensor_reduce(msum[:, g0:g1b + 1], ev,
                                op=ALU.add, axis=mybir.AxisListType.X)


def _gru_step(nc, p, W, pre, x_ap, h_ap):
    """One feature-major GRU cell step; h_ap [32, BL] updated in place.

    x_ap is the pre-GRU relu vector (with ones row); the upstream linear is
    folded into the wx_* gate weights.
    """
    ps_r = p.pssm.tile([H, BL], FP, tag="sm")
    nc.tensor.matmul(ps_r[:], W[f"wx_{pre}_r"][:], x_ap,
                     start=True, stop=False)
    nc.tensor.matmul(ps_r[:], W[f"whh_{pre}_r"][:], h_ap,
                     start=False, stop=True)
    r = p.wk.tile([H, BL], FP, tag="r")
    nc.scalar.activation(r[:], ps_r[:], AF.Sigmoid, bias=W[f"bs_{pre}_r"][:])
    ps_z = p.pssm.tile([H, BL], FP, tag="sm")
    nc.tensor.matmul(ps_z[:], W[f"wx_{pre}_z"][:], x_ap,
                     start=True, stop=False)
    nc.tensor.matmul(ps_z[:], W[f"whh_{pre}_z"][:], h_ap,
                     start=False, stop=True)
    z = p.wk.tile([H, BL], FP, tag="z")
    nc.scalar.activation(z[:], ps_z[:], AF.Sigmoid, bias=W[f"bs_{pre}_z"][:])
    ps_gin = p.pssm.tile([H, BL], FP, tag="sm")
    nc.tensor.matmul(ps_gin[:], W[f"wx_{pre}_n"][:], x_ap,
                     start=True, stop=True)
    ps_ghn = p.pssm.tile([H, BL], FP, tag="sm")
    nc.tensor.matmul(ps_ghn[:], W[f"whh_{pre}_n"][:], h_ap,
                     start=True, stop=True)
    hn = p.wk.tile([H, BL], FP, tag="hn")
    nc.scalar.activation(hn[:], ps_ghn[:], AF.Identity,
                         bias=W[f"bhhn_{pre}"][:])
    rhn = p.wk.tile([H, BL], FP, tag="rhn")
    nc.vector.tensor_mul(rhn[:], r[:], hn[:])
    npre = p.wk.tile([H, BL], FP, tag="npre")
    nc.vector.tensor_add(npre[:], ps_gin[:], rhn[:])
    nt = p.wk.tile([H, BL], FP, tag="nt")
    nc.scalar.activation(nt[:], npre[:], AF.Tanh, bias=W[f"bihn_{pre}"][:])
    hmn = p.wk.tile([H, BL], FP, tag="hmn")
    nc.vector.tensor_sub(hmn[:], h_ap, nt[:])
    zh = p.wk.tile([H, BL], FP, tag="zh")
    nc.vector.tensor_mul(zh[:], z[:], hmn[:])
    nc.vector.tensor_add(h_ap, nt[:], zh[:])


def build_program(wshapes):
    """Build + compile the 8-core SPMD program. wshapes: weight name->shape."""
    nc = bacc.Bacc("TRN2", target_bir_lowering=False, debug=False,
                   num_devices=N_CORES)

    # ---------- DRAM I/O ----------
    Wd = {}
    for name, shp in wshapes.items():
        dt = FP if name in _FP_WEIGHTS else FR
        Wd[name] = nc.dram_tensor(name, list(shp), dt,
                                  kind="ExternalInput").ap()

    xfT_d = nc.dram_tensor("xfT", [TAB, F, K], FR, kind="ExternalInput").ap()
    # all 17 t's of local x, feature-major with ones row: (57, 544)
    xla_d = nc.dram_tensor("xlT_all", [F + 1, TAB * BL], FR,
                           kind="ExternalInput").ap()
    pose0T_full_d = nc.dram_tensor("pose0T_full", [F, K], FR,
                                   kind="ExternalInput").ap()
    pose0T_loc_d = nc.dram_tensor("pose0T_loc", [F, BL], FR,
                                  kind="ExternalInput").ap()
    pose0_loc_b_d = nc.dram_tensor("pose0_loc_b", [BL, F], FP,
                                   kind="ExternalInput").ap()

    poses_out = nc.dram_tensor("poses_loc", [TPRED, BL, F], FP,
                               kind="ExternalOutput").ap()
    stab_out = nc.dram_tensor("stab_loc", [1, BL], FP,
                              kind="ExternalOutput").ap()
    dbg = {}
    if DEBUG:
        for nm, shp in [("dbg_conf", [H, BL]), ("dbg_em0", [H, BL]),
                        ("dbg_es", [H, BL]), ("dbg_ed0", [H, BL])]:
            dbg[nm] = nc.dram_tensor(nm, shp, FR, kind="ExternalOutput").ap()
        dbg["dbg_mask"] = nc.dram_tensor("dbg_mask", [F, BL], FP,
                                         kind="ExternalOutput").ap()

    rg = [list(range(N_CORES))]

    with tile.TileContext(nc) as tc:
        with (
            tc.tile_pool(name="const", bufs=1) as cpool,
            tc.tile_pool(name="state", bufs=1) as st,
            tc.tile_pool(name="xin", bufs=4) as xin,
            tc.tile_pool(name="work", bufs=6) as wk,
            tc.tile_pool(name="blk", bufs=5) as blk,
            tc.tile_pool(name="psV", bufs=2, space="PSUM") as psV_pool,
            tc.tile_pool(name="psmm", bufs=4, space="PSUM") as psmm_pool,
            tc.tile_pool(name="pssm", bufs=2, space="PSUM") as pssm,
            tc.tile_pool(name="dram", bufs=2, space="DRAM") as dram,
        ):
            p = _P()
            p.wk, p.blk, p.pssm = wk, blk, pssm
            p.ps2, p.ps3 = psmm_pool, psmm_pool
            p.zeros = cpool.tile([128, 512], FP, tag="zeros")
            nc.vector.memset(p.zeros[:], 0.0)

            # ---- load constants into SBUF ----
            W = {}
            for name, shp in wshapes.items():
                dt = FP if name in _FP_WEIGHTS else FR
                t = cpool.tile(list(shp), dt, tag=f"c_{name}")
                nc.sync.dma_start(t[:], Wd[name][:])
                W[name] = t

            # ---- persistent state ----
            hconf = st.tile([H, BL], FR, tag="hconf")     # phase-2 GRU state
            nc.vector.tensor_copy(hconf[:], p.zeros[0:H, 0:BL])
            # ginT = xcat^T local: rows 0:56 pose, 56:88 conf, 88 ones
            ginT = st.tile([G + 1, BL], FR, tag="ginT")
            nc.sync.dma_start(ginT[0:F, :], pose0T_loc_d[:])
            nc.sync.dma_start(ginT[G:G + 1, :], Wd["ones_fr"][:, 0:BL])
            # xcT_full: rows 0:32 conf^T full, 32:88 pose0^T full, 88 ones
            xcT = st.tile([G + 1, K], FR, tag="xcT")
            nc.sync.dma_start(xcT[H:G, :], pose0T_full_d[:])
            nc.sync.dma_start(xcT[G:G + 1, :], Wd["ones_fr"][:])
            # xcl = xc^T local: rows 0:32 conf, 32:88 pose, 88 ones
            xcl = st.tile([G + 1, BL], FR, tag="xcl")
            nc.sync.dma_start(xcl[H:G, :], pose0T_loc_d[:])
            nc.sync.dma_start(xcl[G:G + 1, :], Wd["ones_fr"][:, 0:BL])
            # hd_aug: GRU-delta state + ones row
            hd_aug = st.tile([H + 1, BL], FR, tag="hd_aug")
            nc.vector.tensor_copy(hd_aug[0:H, :], p.zeros[0:H, 0:BL])
            nc.sync.dma_start(hd_aug[H:H + 1, :], Wd["ones_fr"][:, 0:BL])
            pose_b = st.tile([BL, F], FP, tag="pose_b")   # b-major pose copy
            nc.sync.dma_start(pose_b[:], pose0_loc_b_d[:])
            mask56 = st.tile([F, BL], FR, tag="mask56")
            # persistent relu tile with ones row (stab head)
            s1 = st.tile([H + 1, BL], FR, tag="s1")
            nc.sync.dma_start(s1[H:H + 1, :], Wd["ones_fr"][:, 0:BL])
            # full xcat^T = [pose; conf] over all 256 objects (phase-4 V side)
            xdT = st.tile([G, K], FR, tag="xdT")
            nc.sync.dma_start(xdT[0:F, :], pose0T_full_d[:])
            # whole local x batch (feature-major + ones rows)
            xla = st.tile([F + 1, TAB * BL], FR, tag="xla")
            nc.sync.dma_start(xla[:], xla_d[:])

            # ============ batched U for phase 1: ub_all [128, 136] ==========
            # U cols are (t, b): col = 32t + b, b = 4g + j; ub col = 8t + g
            ub_all = st.tile([128, TAB * NB], FP, tag="ub_all")
            for c0, c1 in [(0, 256), (256, TAB * BL)]:
                psUh = pssm.tile([H, c1 - c0], FP, tag="sm")
                nc.tensor.matmul(psUh[:], W["w1t_aug"][:], xla[:, c0:c1],
                                 start=True, stop=True)
                sv = psUh[:, :].rearrange("f (g j) -> f g j", j=4)
                gc0 = c0 // 4
                ng = (c1 - c0) // 4
                for j in range(4):
                    if j % 2 == 0:
                        nc.scalar.copy(
                            ub_all[32 * j:32 * j + 32, gc0:gc0 + ng],
                            sv[:, :, j])
                    else:
                        nc.vector.tensor_copy(
                            ub_all[32 * j:32 * j + 32, gc0:gc0 + ng],
                            sv[:, :, j])

            # ================= phase 1 + 2: gcn_on_AB + GRU =================
            for t in range(TAB):
                xf = xin.tile([F, K], FR, tag="xf")
                nc.sync.dma_start(xf[:], xfT_d[t])

                psV = psV_pool.tile([128, K], FP, tag="psV")
                nc.tensor.matmul(psV[:], W["w1b4"][:], xf[:],
                                 start=True, stop=True)
                msum = wk.tile([128, NB], FP, tag="msum")
                _edge_blocks(nc, p, psV[:],
                             lambda g, t=t: ub_all[:, 8 * t + g:8 * t + g + 1],
                             W["w2bd"], W["w3bd"],
                             W["b2_4"][:], W["b3_4"][:], msum)

                emT = wk.tile([H, BL], FR, tag="emT")
                _deinterleave(nc, emT[:, :], msum)

                pso1 = pssm.tile([H, BL], FP, tag="sm")
                nc.tensor.matmul(pso1[:], W["wo1a"][:],
                                 xla[:, t * BL:(t + 1) * BL],
                                 start=True, stop=False)
                nc.tensor.matmul(pso1[:], W["wo1b"][:], emT[:],
                                 start=False, stop=True)
                q1 = wk.tile([H, BL], FR, tag="q1")
                nc.scalar.activation(q1[:], pso1[:], AF.Relu)
                if DEBUG and t == 0:
                    nc.sync.dma_start(dbg["dbg_em0"][:], emT[:])

                _gru_step(nc, p, W, "r", q1[:], hconf[:])

            # conf into ginT/xcl (SBUF->SBUF DMA handles row offsets)
            nc.sync.dma_start(ginT[F:G, :], hconf[:])
            nc.sync.dma_start(xcl[0:H, :], hconf[:])
            if DEBUG:
                nc.sync.dma_start(dbg["dbg_conf"][:], hconf[:])

            # ================= conf AllGather =================
            cin = dram.tile([H, BL], FR, tag="cin")
            nc.sync.dma_start(cin[:], hconf[:])
            cout = dram.tile([K, BL], FR, tag="cout")
            nc.gpsimd.collective_compute(
                "AllGather", ALU.bypass, replica_groups=rg,
                ins=[cin.opt()], outs=[cout.opt()])
            cview = cout[:, :].rearrange("(r f) b -> f r b", f=H)
            nc.sync.dma_start(
                xcT[0:H, :].rearrange("f (r b) -> f r b", b=BL), cview)
            nc.sync.dma_start(
                xdT[F:G, :].rearrange("f (r b) -> f r b", b=BL), cview)

            # ================= phase 3: pred_stab =================
            psUs = pssm.tile([H, BL], FP, tag="sm")
            nc.tensor.matmul(psUs[:], W["w1st_aug"][:], xcl[:],
                             start=True, stop=True)
            ubs = _interleave(nc, p, psUs[:, :], NB)
            psVs = psV_pool.tile([128, K], FP, tag="psV")
            nc.tensor.matmul(psVs[:], W["w1sb4"][:], xcT[0:G, :],
                             start=True, stop=True)
            msums = wk.tile([128, NB], FP, tag="msum")
            _edge_blocks(nc, p, psVs[:], lambda g: ubs[:, g:g + 1],
                         W["w2sbd"], W["w3sbd"],
                         W["b2s_4"][:], W["b3s_4"][:], msums)
            esT = wk.tile([H, BL], FR, tag="esT")
            _deinterleave(nc, esT[:, :], msums)
            if DEBUG:
                nc.sync.dma_start(dbg["dbg_es"][:], esT[:])

            pss1 = pssm.tile([H, BL], FP, tag="sm")
            nc.tensor.matmul(pss1[:], W["ws1a"][:], xcl[:],
                             start=True, stop=False)
            nc.tensor.matmul(pss1[:], W["ws1b"][:], esT[:],
                             start=False, stop=True)
            nc.scalar.activation(s1[0:H, :], pss1[:], AF.Relu)
            pss2 = pssm.tile([1, BL], FP, tag="sm")
            nc.tensor.matmul(pss2[:], W["ws2_aug"][:], s1[:],
                             start=True, stop=True)
            stab_sb = wk.tile([1, BL], FP, tag="stab_sb")
            nc.scalar.copy(stab_sb[:], pss2[:])
            nc.sync.dma_start(stab_out[:], stab_sb[:])
            # mask row: 1.0 where stab <= 0
            maskr = wk.tile([1, BL], FR, tag="maskr")
            nc.vector.tensor_scalar(maskr[:], pss2[:], 0.0, None,
                                    op0=ALU.is_le)
            psm = pssm.tile([F, BL], FP, tag="sm")
            nc.tensor.matmul(psm[:], W["ones_1x56"][:], maskr[:],
                             start=True, stop=True)
            nc.vector.tensor_copy(mask56[:], psm[:])
            if DEBUG:
                mask56fp = wk.tile([F, BL], FP, tag="mask56fp")
                nc.vector.tensor_copy(mask56fp[:], psm[:])
                nc.sync.dma_start(dbg["dbg_mask"][:], mask56fp[:])

            # ================= phase 4: delta loop =================
            for i in range(TPRED):
                psV4 = psV_pool.tile([128, K], FP, tag="psV")
                nc.tensor.matmul(psV4[:], W["w1db4"][:], xdT[:],
                                 start=True, stop=True)
                psUd = pssm.tile([H, BL], FP, tag="sm")
                nc.tensor.matmul(psUd[:], W["w1dt_aug"][:], ginT[:],
                                 start=True, stop=True)
                ubd = _interleave(nc, p, psUd[:, :], NB)
                msumd = wk.tile([128, NB], FP, tag="msum")
                _edge_blocks(nc, p, psV4[:], lambda g: ubd[:, g:g + 1],
                             W["w2dbd"], W["w3dbd"],
                             W["b2d_4"][:], W["b3d_4"][:], msumd)
                edT = wk.tile([H, BL], FR, tag="edT")
                _deinterleave(nc, edT[:, :], msumd)
                if DEBUG and i == 0:
                    nc.sync.dma_start(dbg["dbg_ed0"][:], edT[:])

                psg1 = pssm.tile([H, BL], FP, tag="sm")
                nc.tensor.matmul(psg1[:], W["wg1a"][:], ginT[:],
                                 start=True, stop=False)
                nc.tensor.matmul(psg1[:], W["wg1b"][:], edT[:],
                                 start=False, stop=True)
                g1 = wk.tile([H, BL], FR, tag="g1")
                nc.scalar.activation(g1[:], psg1[:], AF.Relu)

                _gru_step(nc, p, W, "rd", g1[:], hd_aug[0:H, :])

                psd = pssm.tile([F, BL], FP, tag="sm")
                nc.tensor.matmul(psd[:], W["wfc_aug"][:], hd_aug[:],
                                 start=True, stop=True)
                delta = wk.tile([F, BL], FR, tag="delta")
                nc.vector.tensor_mul(delta[:], psd[:], mask56[:])
                # pose update (feature-major, in place)
                nc.vector.tensor_add(ginT[0:F, :], ginT[0:F, :], delta[:])

                # b-major pose snapshot -> DRAM output
                psdT = pssm.tile([BL, F], FR, tag="sm")
                nc.tensor.transpose(psdT[:], delta[:], W["ident56"][:])
                nc.vector.tensor_add(pose_b[:], pose_b[:], psdT[:])
                nc.sync.dma_start(poses_out[i], pose_b[:])

                if i < TPRED - 1:
                    # all-gather this iteration's delta; update full xcat
                    din = dram.tile([F, BL], FR, tag="din")
                    nc.sync.dma_start(din[:], delta[:])
                    dout = dram.tile([N_CORES * F, BL], FR, tag="dout")
                    nc.gpsimd.collective_compute(
                        "AllGather", ALU.bypass, replica_groups=rg,
                        ins=[din.opt()], outs=[dout.opt()])
                    dfull = wk.tile([F, K], FR, tag="dfull")
                    dv = dout[:, :].rearrange("(r f) b -> f r b", f=F)
                    nc.sync.dma_start(
                        dfull[:, :].rearrange("f (r b) -> f r b", b=BL), dv)
                    nc.vector.tensor_add(xdT[0:F, :], xdT[0:F, :], dfull[:])

    nc.compile()
    return nc


_CACHE = {}


def kernel(struct_obs_ab, struct_obs_c, params):
    x_ab = _np(struct_obs_ab)            # (17, 256, 56)
    pose0 = _np(struct_obs_c)[0]         # (256, 56)

    wd = _prep_weights(params)
    wshapes = {k: v.shape for k, v in wd.items()}

    if "prog" not in _CACHE:
        _CACHE["prog"] = build_program(wshapes)
    nc = _CACHE["prog"]

    xfT = np.ascontiguousarray(x_ab.transpose(0, 2, 1))   # (17, 56, 256)
    pose0T = np.ascontiguousarray(pose0.T)                # (56, 256)

    in_maps = []
    for c in range(N_CORES):
        sl = slice(c * BL, (c + 1) * BL)
        # (57, 17*32): col 32t+b = [x_ab[t, local b]; 1]
        xla = np.concatenate(
            [x_ab[:, sl, :].transpose(0, 2, 1),
             np.ones((TAB, 1, BL), np.float32)], axis=1)   # (17, 57, 32)
        xla = np.ascontiguousarray(
            xla.transpose(1, 0, 2).reshape(F + 1, TAB * BL))
        m = dict(wd)
        m["xfT"] = xfT
        m["xlT_all"] = xla
        m["pose0T_full"] = pose0T
        m["pose0T_loc"] = np.ascontiguousarray(pose0T[:, sl])
        m["pose0_loc_b"] = np.ascontiguousarray(pose0[sl, :])
        in_maps.append(m)

    res = run_bass_kernel_spmd(nc, in_maps, core_ids=list(range(N_CORES)))
    _CACHE["last_results"] = res

    poses = np.zeros((1, TPRED, K, F), np.float32)
    stab = np.zeros((1, K), np.float32)
    for c in range(N_CORES):
        sl = slice(c * BL, (c + 1) * BL)
        poses[0, :, sl, :] = res.results[c]["poses_loc"]
        stab[0, sl] = res.results[c]["stab_loc"][0]

    stability = np.broadcast_to(stab[:, None, :], (1, TPRED, K)).copy()
    return poses, stability


# revision 21
# speedup vs baseline: 1.1794x; 1.0560x over previous
"""CoPhyNet Trainium2 kernel — 8-core SPMD Bass/Tile implementation.

Self-contained: hardcodes shapes from the problem spec.
  struct_obs_ab: (17, 256, 56) fp32
  struct_obs_c:  (1, 256, 56) fp32

Sharding: the object axis K=256 is split 8 ways (32 "local" objects per
core). All-pairs edge MLPs: pair[p,q] = concat(x[q], x[p]), output index q
(local), mean over p (free axis). Layer 1 is decomposed into an outer sum
U[q] + V[p]; V is streamed over all 256 p as the matmul moving operand,
U enters as the per-partition activation bias. Layers 2/3 run as 4-way
block-diagonal [128,128] @ [128,256] float32r matmuls (4 local objects
packed in the partition dim). The delta loop all-gathers each core's
32-row V contribution (4 KB) per iteration.
"""

import numpy as np

import concourse.bass as bass
import concourse.bacc as bacc
import concourse.tile as tile
import concourse.mybir as mybir
from concourse.bass_utils import run_bass_kernel_spmd

FP = mybir.dt.float32
FR = mybir.dt.float32r
AF = mybir.ActivationFunctionType
ALU = mybir.AluOpType

N_CORES = 8
TAB = 17
K = 256
F = 56
H = 32
BL = K // N_CORES          # local objects per core = 32
NB = BL // 4               # 4-packed blocks per core = 8
TPRED = TAB - 1
G = H + F                  # 88
DEBUG = False


def _np(x):
    return np.asarray(x, dtype=np.float32)


def _blockdiag4(w):
    out = np.zeros((128, 128), dtype=np.float32)
    for j in range(4):
        out[32 * j:32 * j + 32, 32 * j:32 * j + 32] = w
    return out


def _prep_weights(params):
    """Host-side weight preprocessing -> dict of np arrays (DRAM inputs)."""
    d = {}

    def lin(p):
        return _np(p["w"]), _np(p["b"])

    # ---- phase 1: mlp_inter (112->32->32->32) ----
    w1, b1 = lin(params["mlp_inter"][0])
    w2, b2 = lin(params["mlp_inter"][1])
    w3, b3 = lin(params["mlp_inter"][2])
    d["w1t_aug"] = np.concatenate([w1[:F], b1[None, :]], 0)          # (57, 32)
    d["w1b4"] = np.tile(w1[F:], (1, 4))                               # (56, 128)
    d["w2bd"] = _blockdiag4(w2)                                       # (128, 128)
    d["w3bd"] = _blockdiag4(w3)
    d["b2_4"] = np.tile(b2, 4)[:, None]                               # (128, 1)
    d["b3_4"] = np.tile(b3, 4)[:, None]

    # ---- mlp_out (88->32->32), E rows prescaled by 1/K ----
    wo1, bo1 = lin(params["mlp_out"][0])
    wo2, bo2 = lin(params["mlp_out"][1])
    d["wo1a"] = np.concatenate([wo1[:F], bo1[None, :]], 0)            # (57, 32)
    d["wo1b"] = wo1[F:] / K                                           # (32, 32)


    # ---- GRUs: split gate weights; x-side fused with the upstream linear
    # (gates = W_ih^T @ (Wup^T @ v) = (Wup @ W_ih)^T @ v, exact) ----
    _gru_raw = {}
    for name, p in [("r", params["rnn"]), ("rd", params["rnn_delta"])]:
        wih, whh = _np(p["w_ih"]), _np(p["w_hh"])
        bih, bhh = _np(p["b_ih"]), _np(p["b_hh"])
        _gru_raw[name] = wih
        for gi, gn in enumerate(("r", "z", "n")):
            d[f"whh_{name}_{gn}"] = whh[:, gi * H:(gi + 1) * H]
        bs = bih + bhh
        d[f"bs_{name}_r"] = bs[0:H, None]                             # (32, 1)
        d[f"bs_{name}_z"] = bs[H:2 * H, None]
        d[f"bhhn_{name}"] = bhh[2 * H:, None]
        d[f"bihn_{name}"] = bih[2 * H:, None]

    # ---- phase 3: mlp_inter_stab (176->32->32->32), xc = [conf, pose] ----
    ws1, bs1 = lin(params["mlp_inter_stab"][0])
    ws2, bs2 = lin(params["mlp_inter_stab"][1])
    ws3, bs3 = lin(params["mlp_inter_stab"][2])
    d["w1st_aug"] = np.concatenate([ws1[:G], bs1[None, :]], 0)        # (89, 32)
    d["w1sb4"] = np.tile(ws1[G:], (1, 4))                             # (88, 128)
    d["w2sbd"] = _blockdiag4(ws2)
    d["w3sbd"] = _blockdiag4(ws3)
    d["b2s_4"] = np.tile(bs2, 4)[:, None]
    d["b3s_4"] = np.tile(bs3, 4)[:, None]

    # ---- mlp_stab (120->32->1), Es rows prescaled ----
    wm1, bm1 = lin(params["mlp_stab"][0])
    wm2, bm2 = lin(params["mlp_stab"][1])
    d["ws1a"] = np.concatenate([wm1[:G], bm1[None, :]], 0)            # (89, 32)
    d["ws1b"] = wm1[G:] / K                                           # (32, 32)
    d["ws2_aug"] = np.concatenate([wm2, bm2[None, :]], 0)             # (33, 1)

    # ---- phase 4: mlp_inter_delta (176->...), xcat = [pose, conf] ----
    wd1, bd1 = lin(params["mlp_inter_delta"][0])
    wd2, bd2 = lin(params["mlp_inter_delta"][1])
    wd3, bd3 = lin(params["mlp_inter_delta"][2])
    d["w1dt_aug"] = np.concatenate([wd1[:G], bd1[None, :]], 0)        # (89, 32)
    d["w1db4"] = np.tile(wd1[G:], (1, 4))                             # (88, 128)
    d["w2dbd"] = _blockdiag4(wd2)
    d["w3dbd"] = _blockdiag4(wd3)
    d["b2d_4"] = np.tile(bd2, 4)[:, None]
    d["b3d_4"] = np.tile(bd3, 4)[:, None]

    # ---- mlp_gcn_delta (120->32->32), Ed rows prescaled ----
    wg1, bg1 = lin(params["mlp_gcn_delta"][0])
    wg2, bg2 = lin(params["mlp_gcn_delta"][1])
    d["wg1a"] = np.concatenate([wg1[:G], bg1[None, :]], 0)            # (89, 32)
    d["wg1b"] = wg1[G:] / K                                           # (32, 32)
    for gi, gn in enumerate(("r", "z", "n")):
        d[f"wx_r_{gn}"] = wo2 @ _gru_raw["r"][:, gi * H:(gi + 1) * H]
        d[f"wx_rd_{gn}"] = wg2 @ _gru_raw["rd"][:, gi * H:(gi + 1) * H]
    # fold the upstream linear's bias through the gate weights
    for nm, bias in [("r", bo2), ("rd", bg2)]:
        wih = _gru_raw[nm]
        d[f"bs_{nm}_r"] = d[f"bs_{nm}_r"] + (bias @ wih[:, 0:H])[:, None]
        d[f"bs_{nm}_z"] = d[f"bs_{nm}_z"] + (bias @ wih[:, H:2 * H])[:, None]
        d[f"bihn_{nm}"] = d[f"bihn_{nm}"] + (bias @ wih[:, 2 * H:])[:, None]

    # ---- fc_delta (32->56) ----
    wf, bf = lin(params["fc_delta"])
    d["wfc_aug"] = np.concatenate([wf, bf[None, :]], 0)               # (33, 56)

    d["ident56"] = np.eye(F, dtype=np.float32)                        # (56, 56)
    d["ones_1x56"] = np.ones((1, F), dtype=np.float32)                # (1, 56)
    d["ones_fr"] = np.ones((1, K), dtype=np.float32)                  # (1, 256)
    return d


# everything that feeds a matmul is float32r (single-pass PE); fp32 only for
# bias columns (activation bias / tensor_scalar operands)
_FP_WEIGHTS = {
    "b2_4", "b3_4", "b2s_4", "b3s_4", "b2d_4", "b3d_4",
    "bs_r_r", "bs_r_z", "bhhn_r", "bihn_r",
    "bs_rd_r", "bs_rd_z", "bhhn_rd", "bihn_rd",
}


class _P:
    """Pools holder."""


def _interleave(nc, p, psU_ap, n_groups):
    """ub[32j+f, g] = U[f, 4g+j]; psU_ap [32, 4*n_groups] PSUM -> SBUF ub."""
    ub = p.wk.tile([128, n_groups], FP, tag="ub")
    sv = psU_ap.rearrange("f (g j) -> f g j", j=4)
    for j in range(4):
        if j % 2 == 0:
            nc.scalar.copy(ub[32 * j:32 * j + 32, :], sv[:, :, j])
        else:
            nc.vector.tensor_copy(ub[32 * j:32 * j + 32, :], sv[:, :, j])
    return ub


def _deinterleave(nc, dst_ap, src):
    """dst[f, 4g+j] = src[32j+f, g]; dst AP [32, 32] SBUF, src [128, 8]."""
    dv = dst_ap.rearrange("f (g j) -> f g j", j=4)
    for j in range(4):
        if j % 2 == 0:
            nc.scalar.copy(dv[:, :, j], src[32 * j:32 * j + 32, :])
        else:
            nc.vector.tensor_copy(dv[:, :, j], src[32 * j:32 * j + 32, :])


def _edge_blocks(nc, p, v4_ap, ub_cols, w2bd, w3bd, b2col, b3col, msum):
    """8 blocks of the 4-packed edge MLP, processed as 4 block-PAIRS with
    [128, 512] matmuls/passes; msum [128, 8] gets per-block sums.

    ub_cols(g) -> [128, 1] bias AP for block g.
    """
    for pr in range(NB // 2):
        g0, g1b = 2 * pr, 2 * pr + 1
        h1 = p.blk.tile([128, 512], FR, tag="h1")
        nc.scalar.activation(h1[:, 0:256], v4_ap, AF.Relu, bias=ub_cols(g0))
        nc.vector.scalar_tensor_tensor(h1[:, 256:512], v4_ap, ub_cols(g1b),
                                       p.zeros[:, 0:256],
                                       op0=ALU.add, op1=ALU.max)
        ps2 = p.ps2.tile([128, 512], FP, tag="mm")
        nc.tensor.matmul(ps2[:], w2bd[:], h1[:], start=True, stop=True)
        h2 = p.blk.tile([128, 512], FR, tag="h2")
        if pr % 2 == 0:
            nc.vector.tensor_scalar(h2[:], ps2[:], b2col, 0.0,
                                    op0=ALU.add, op1=ALU.max)
        else:
            nc.scalar.activation(h2[:], ps2[:], AF.Relu, bias=b2col)
        ps3 = p.ps3.tile([128, 512], FP, tag="mm")
        nc.tensor.matmul(ps3[:], w3bd[:], h2[:], start=True, stop=True)
        e3 = p.blk.tile([128, 512], FP, tag="e3")
        if pr % 2 == 0:
            # DVE: relu pass + strided free-axis reduce
            nc.vector.scalar_tensor_tensor(e3[:], ps3[:], b3col, p.zeros[:],
                                           op0=ALU.add, op1=ALU.max)
            ev = e3[:, :].rearrange("q (pair a) -> q pair a", pair=2)
            nc.vector.tensor_reduce(msum[:, g0:g1b + 1], ev,
                                    op=ALU.add, axis=mybir.AxisListType.X)
        else:
            # ACT: two relu halves with fused accumulators
            nc.scalar.activation(e3[:, 0:256], ps3[:, 0:256], AF.Relu,
                                 bias=b3col, accum_out=msum[:, g0:g0 + 1])
            nc.scalar.activation(e3[:, 256:512], ps3[:, 256:512], AF.Relu,
                                 bias=b3col, accum_out=msum[:, g1b:g1b + 1])


def _gru_step(nc, p, W, pre, x_ap, h_ap):
    """One feature-major GRU cell step; h_ap [32, BL] updated in place.

    x_ap is the pre-GRU relu vector (with ones row); the upstream linear is
    folded into the wx_* gate weights.
    """
    ps_r = p.pssm.tile([H, BL], FP, tag="sm")
    nc.tensor.matmul(ps_r[:], W[f"wx_{pre}_r"][:], x_ap,
                     start=True, stop=False)
    nc.tensor.matmul(ps_r[:], W[f"whh_{pre}_r"][:], h_ap,
                     start=False, stop=True)
    r = p.wk.tile([H, BL], FP, tag="r")
    nc.scalar.activation(r[:], ps_r[:], AF.Sigmoid, bias=W[f"bs_{pre}_r"][:])
    ps_z = p.pssm.tile([H, BL], FP, tag="sm")
    nc.tensor.matmul(ps_z[:], W[f"wx_{pre}_z"][:], x_ap,
                     start=True, stop=False)
    nc.tensor.matmul(ps_z[:], W[f"whh_{pre}_z"][:], h_ap,
                     start=False, stop=True)
    z = p.wk.tile([H, BL], FP, tag="z")
    nc.scalar.activation(z[:], ps_z[:], AF.Sigmoid, bias=W[f"bs_{pre}_z"][:])
    ps_gin = p.pssm.tile([H, BL], FP, tag="sm")
    nc.tensor.matmul(ps_gin[:], W[f"wx_{pre}_n"][:], x_ap,
                     start=True, stop=True)
    ps_ghn = p.pssm.tile([H, BL], FP, tag="sm")
    nc.tensor.matmul(ps_ghn[:], W[f"whh_{pre}_n"][:], h_ap,
                     start=True, stop=True)
    hn = p.wk.tile([H, BL], FP, tag="hn")
    nc.scalar.activation(hn[:], ps_ghn[:], AF.Identity,
                         bias=W[f"bhhn_{pre}"][:])
    rhn = p.wk.tile([H, BL], FP, tag="rhn")
    nc.vector.tensor_mul(rhn[:], r[:], hn[:])
    npre = p.wk.tile([H, BL], FP, tag="npre")
    nc.vector.tensor_add(npre[:], ps_gin[:], rhn[:])
    nt = p.wk.tile([H, BL], FP, tag="nt")
    nc.scalar.activation(nt[:], npre[:], AF.Tanh, bias=W[f"bihn_{pre}"][:])
    hmn = p.wk.tile([H, BL], FP, tag="hmn")
    nc.vector.tensor_sub(hmn[:], h_ap, nt[:])
    zh = p.wk.tile([H, BL], FP, tag="zh")
    nc.vector.tensor_mul(zh[:], z[:], hmn[:])
    nc.vector.tensor_add(h_ap, nt[:], zh[:])


def build_program(wshapes):
    """Build + compile the 8-core SPMD program. wshapes: weight name->shape."""
    nc = bacc.Bacc("TRN2", target_bir_lowering=False, debug=False,
                   num_devices=N_CORES)

    # ---------- DRAM I/O ----------
    Wd = {}
    for name, shp in wshapes.items():
        dt = FP if name in _FP_WEIGHTS else FR
        Wd[name] = nc.dram_tensor(name, list(shp), dt,
                                  kind="ExternalInput").ap()

    xfT_d = nc.dram_tensor("xfT", [TAB, F, K], FR, kind="ExternalInput").ap()
    # all 17 t's of local x, feature-major with ones row: (57, 544)
    xla_d = nc.dram_tensor("xlT_all", [F + 1, TAB * BL], FR,
                           kind="ExternalInput").ap()
    pose0T_full_d = nc.dram_tensor("pose0T_full", [F, K], FR,
                                   kind="ExternalInput").ap()
    pose0T_loc_d = nc.dram_tensor("pose0T_loc", [F, BL], FR,
                                  kind="ExternalInput").ap()
    pose0_loc_b_d = nc.dram_tensor("pose0_loc_b", [BL, F], FP,
                                   kind="ExternalInput").ap()

    poses_out = nc.dram_tensor("poses_loc", [TPRED, BL, F], FP,
                               kind="ExternalOutput").ap()
    stab_out = nc.dram_tensor("stab_loc", [1, BL], FP,
                              kind="ExternalOutput").ap()
    dbg = {}
    if DEBUG:
        for nm, shp in [("dbg_conf", [H, BL]), ("dbg_em0", [H, BL]),
                        ("dbg_es", [H, BL]), ("dbg_ed0", [H, BL])]:
            dbg[nm] = nc.dram_tensor(nm, shp, FR, kind="ExternalOutput").ap()
        dbg["dbg_mask"] = nc.dram_tensor("dbg_mask", [F, BL], FP,
                                         kind="ExternalOutput").ap()

    rg = [list(range(N_CORES))]

    with tile.TileContext(nc) as tc:
        with (
            tc.tile_pool(name="const", bufs=1) as cpool,
            tc.tile_pool(name="state", bufs=1) as st,
            tc.tile_pool(name="xin", bufs=4) as xin,
            tc.tile_pool(name="work", bufs=8) as wk,
            tc.tile_pool(name="blk", bufs=6) as blk,
            tc.tile_pool(name="psV", bufs=2, space="PSUM") as psV_pool,
            tc.tile_pool(name="psmm", bufs=4, space="PSUM") as psmm_pool,
            tc.tile_pool(name="pssm", bufs=2, space="PSUM") as pssm,
            tc.tile_pool(name="dram", bufs=2, space="DRAM") as dram,
        ):
            p = _P()
            p.wk, p.blk, p.pssm = wk, blk, pssm
            p.ps2, p.ps3 = psmm_pool, psmm_pool
            p.zeros = cpool.tile([128, 512], FP, tag="zeros")
            nc.vector.memset(p.zeros[:], 0.0)

            # ---- load constants into SBUF ----
            W = {}
            for name, shp in wshapes.items():
                dt = FP if name in _FP_WEIGHTS else FR
                t = cpool.tile(list(shp), dt, tag=f"c_{name}")
                nc.sync.dma_start(t[:], Wd[name][:])
                W[name] = t

            # ---- persistent state ----
            hconf = st.tile([H, BL], FR, tag="hconf")     # phase-2 GRU state
            nc.vector.tensor_copy(hconf[:], p.zeros[0:H, 0:BL])
            # ginT = xcat^T local: rows 0:56 pose, 56:88 conf, 88 ones
            ginT = st.tile([G + 1, BL], FR, tag="ginT")
            nc.sync.dma_start(ginT[0:F, :], pose0T_loc_d[:])
            nc.sync.dma_start(ginT[G:G + 1, :], Wd["ones_fr"][:, 0:BL])
            # xcT_full: rows 0:32 conf^T full, 32:88 pose0^T full, 88 ones
            xcT = st.tile([G + 1, K], FR, tag="xcT")
            nc.sync.dma_start(xcT[H:G, :], pose0T_full_d[:])
            nc.sync.dma_start(xcT[G:G + 1, :], Wd["ones_fr"][:])
            # xcl = xc^T local: rows 0:32 conf, 32:88 pose, 88 ones
            xcl = st.tile([G + 1, BL], FR, tag="xcl")
            nc.sync.dma_start(xcl[H:G, :], pose0T_loc_d[:])
            nc.sync.dma_start(xcl[G:G + 1, :], Wd["ones_fr"][:, 0:BL])
            # hd_aug: GRU-delta state + ones row
            hd_aug = st.tile([H + 1, BL], FR, tag="hd_aug")
            nc.vector.tensor_copy(hd_aug[0:H, :], p.zeros[0:H, 0:BL])
            nc.sync.dma_start(hd_aug[H:H + 1, :], Wd["ones_fr"][:, 0:BL])
            pose_b = st.tile([BL, F], FP, tag="pose_b")   # b-major pose copy
            nc.sync.dma_start(pose_b[:], pose0_loc_b_d[:])
            mask56 = st.tile([F, BL], FR, tag="mask56")
            # persistent relu tile with ones row (stab head)
            s1 = st.tile([H + 1, BL], FR, tag="s1")
            nc.sync.dma_start(s1[H:H + 1, :], Wd["ones_fr"][:, 0:BL])
            # full xcat^T = [pose; conf] over all 256 objects (phase-4 V side)
            xdT = st.tile([G, K], FR, tag="xdT")
            nc.sync.dma_start(xdT[0:F, :], pose0T_full_d[:])
            # whole local x batch (feature-major + ones rows)
            xla = st.tile([F + 1, TAB * BL], FR, tag="xla")
            nc.sync.dma_start(xla[:], xla_d[:])

            # ============ batched U for phase 1: ub_all [128, 136] ==========
            # U cols are (t, b): col = 32t + b, b = 4g + j; ub col = 8t + g
            ub_all = st.tile([128, TAB * NB], FP, tag="ub_all")
            for c0, c1 in [(0, 256), (256, TAB * BL)]:
                psUh = pssm.tile([H, c1 - c0], FP, tag="sm")
                nc.tensor.matmul(psUh[:], W["w1t_aug"][:], xla[:, c0:c1],
                                 start=True, stop=True)
                sv = psUh[:, :].rearrange("f (g j) -> f g j", j=4)
                gc0 = c0 // 4
                ng = (c1 - c0) // 4
                for j in range(4):
                    if j % 2 == 0:
                        nc.scalar.copy(
                            ub_all[32 * j:32 * j + 32, gc0:gc0 + ng],
                            sv[:, :, j])
                    else:
                        nc.vector.tensor_copy(
                            ub_all[32 * j:32 * j + 32, gc0:gc0 + ng],
                            sv[:, :, j])

            # ================= phase 1 + 2: gcn_on_AB + GRU =================
            for t in range(TAB):
                xf = xin.tile([F, K], FR, tag="xf")
                nc.sync.dma_start(xf[:], xfT_d[t])

                psV = psV_pool.tile([128, K], FP, tag="psV")
                nc.tensor.matmul(psV[:], W["w1b4"][:], xf[:],
                                 start=True, stop=True)
                msum = wk.tile([128, NB], FP, tag="msum")
                _edge_blocks(nc, p, psV[:],
                             lambda g, t=t: ub_all[:, 8 * t + g:8 * t + g + 1],
                             W["w2bd"], W["w3bd"],
                             W["b2_4"][:], W["b3_4"][:], msum)

                emT = wk.tile([H, BL], FR, tag="emT")
                _deinterleave(nc, emT[:, :], msum)

                pso1 = pssm.tile([H, BL], FP, tag="sm")
                nc.tensor.matmul(pso1[:], W["wo1a"][:],
                                 xla[:, t * BL:(t + 1) * BL],
                                 start=True, stop=False)
                nc.tensor.matmul(pso1[:], W["wo1b"][:], emT[:],
                                 start=False, stop=True)
                q1 = wk.tile([H, BL], FR, tag="q1")
                nc.scalar.activation(q1[:], pso1[:], AF.Relu)
                if DEBUG and t == 0:
                    nc.sync.dma_start(dbg["dbg_em0"][:], emT[:])

                _gru_step(nc, p, W, "r", q1[:], hconf[:])

            # conf into ginT/xcl (SBUF->SBUF DMA handles row offsets)
            nc.sync.dma_start(ginT[F:G, :], hconf[:])
            nc.sync.dma_start(xcl[0:H, :], hconf[:])
            if DEBUG:
                nc.sync.dma_start(dbg["dbg_conf"][:], hconf[:])

            # ================= conf AllGather =================
            cin = dram.tile([H, BL], FR, tag="cin")
            nc.sync.dma_start(cin[:], hconf[:])
            cout = dram.tile([K, BL], FR, tag="cout")
            nc.gpsimd.collective_compute(
                "AllGather", ALU.bypass, replica_groups=rg,
                ins=[cin.opt()], outs=[cout.opt()])
            cview = cout[:, :].rearrange("(r f) b -> f r b", f=H)
            nc.sync.dma_start(
                xcT[0:H, :].rearrange("f (r b) -> f r b", b=BL), cview)
            nc.sync.dma_start(
                xdT[F:G, :].rearrange("f (r b) -> f r b", b=BL), cview)

            # ================= phase 3: pred_stab =================
            psUs = pssm.tile([H, BL], FP, tag="sm")
            nc.tensor.matmul(psUs[:], W["w1st_aug"][:], xcl[:],
                             start=True, stop=True)
            ubs = _interleave(nc, p, psUs[:, :], NB)
            psVs = psV_pool.tile([128, K], FP, tag="psV")
            nc.tensor.matmul(psVs[:], W["w1sb4"][:], xcT[0:G, :],
                             start=True, stop=True)
            msums = wk.tile([128, NB], FP, tag="msum")
            _edge_blocks(nc, p, psVs[:], lambda g: ubs[:, g:g + 1],
                         W["w2sbd"], W["w3sbd"],
                         W["b2s_4"][:], W["b3s_4"][:], msums)
            esT = wk.tile([H, BL], FR, tag="esT")
            _deinterleave(nc, esT[:, :], msums)
            if DEBUG:
                nc.sync.dma_start(dbg["dbg_es"][:], esT[:])

            pss1 = pssm.tile([H, BL], FP, tag="sm")
            nc.tensor.matmul(pss1[:], W["ws1a"][:], xcl[:],
                             start=True, stop=False)
            nc.tensor.matmul(pss1[:], W["ws1b"][:], esT[:],
                             start=False, stop=True)
            nc.scalar.activation(s1[0:H, :], pss1[:], AF.Relu)
            pss2 = pssm.tile([1, BL], FP, tag="sm")
            nc.tensor.matmul(pss2[:], W["ws2_aug"][:], s1[:],
                             start=True, stop=True)
            stab_sb = wk.tile([1, BL], FP, tag="stab_sb")
            nc.scalar.copy(stab_sb[:], pss2[:])
            nc.sync.dma_start(stab_out[:], stab_sb[:])
            # mask row: 1.0 where stab <= 0
            maskr = wk.tile([1, BL], FR, tag="maskr")
            nc.vector.tensor_scalar(maskr[:], pss2[:], 0.0, None,
                                    op0=ALU.is_le)
            psm = pssm.tile([F, BL], FP, tag="sm")
            nc.tensor.matmul(psm[:], W["ones_1x56"][:], maskr[:],
                             start=True, stop=True)
            nc.vector.tensor_copy(mask56[:], psm[:])
            if DEBUG:
                mask56fp = wk.tile([F, BL], FP, tag="mask56fp")
                nc.vector.tensor_copy(mask56fp[:], psm[:])
                nc.sync.dma_start(dbg["dbg_mask"][:], mask56fp[:])

            # ================= phase 4: delta loop =================
            for i in range(TPRED):
                psV4 = psV_pool.tile([128, K], FP, tag="psV")
                nc.tensor.matmul(psV4[:], W["w1db4"][:], xdT[:],
                                 start=True, stop=True)
                psUd = pssm.tile([H, BL], FP, tag="sm")
                nc.tensor.matmul(psUd[:], W["w1dt_aug"][:], ginT[:],
                                 start=True, stop=True)
                ubd = _interleave(nc, p, psUd[:, :], NB)
                msumd = wk.tile([128, NB], FP, tag="msum")
                _edge_blocks(nc, p, psV4[:], lambda g: ubd[:, g:g + 1],
                             W["w2dbd"], W["w3dbd"],
                             W["b2d_4"][:], W["b3d_4"][:], msumd)
                edT = wk.tile([H, BL], FR, tag="edT")
                _deinterleave(nc, edT[:, :], msumd)
                if DEBUG and i == 0:
                    nc.sync.dma_start(dbg["dbg_ed0"][:], edT[:])

                psg1 = pssm.tile([H, BL], FP, tag="sm")
                nc.tensor.matmul(psg1[:], W["wg1a"][:], ginT[:],
                                 start=True, stop=False)
                nc.tensor.matmul(psg1[:], W["wg1b"][:], edT[:],
                                 start=False, stop=True)
                g1 = wk.tile([H, BL], FR, tag="g1")
                nc.scalar.activation(g1[:], psg1[:], AF.Relu)

                _gru_step(nc, p, W, "rd", g1[:], hd_aug[0:H, :])

                psd = pssm.tile([F, BL], FP, tag="sm")
                nc.tensor.matmul(psd[:], W["wfc_aug"][:], hd_aug[:],
                                 start=True, stop=True)
                delta = wk.tile([F, BL], FR, tag="delta")
                nc.vector.tensor_mul(delta[:], psd[:], mask56[:])
                # pose update (feature-major, in place)
                nc.vector.tensor_add(ginT[0:F, :], ginT[0:F, :], delta[:])

                # b-major pose snapshot -> DRAM output
                psdT = pssm.tile([BL, F], FR, tag="sm")
                nc.tensor.transpose(psdT[:], delta[:], W["ident56"][:])
                nc.vector.tensor_add(pose_b[:], pose_b[:], psdT[:])
                nc.sync.dma_start(poses_out[i], pose_b[:])

                if i < TPRED - 1:
                    # all-gather this iteration's delta; update full xcat
                    din = dram.tile([F, BL], FR, tag="din")
                    nc.sync.dma_start(din[:], delta[:])
                    dout = dram.tile([N_CORES * F, BL], FR, tag="dout")
                    nc.gpsimd.collective_compute(
                        "AllGather", ALU.bypass, replica_groups=rg,
                        ins=[din.opt()], outs=[dout.opt()])
                    dfull = wk.tile([F, K], FR, tag="dfull")
                    dv = dout[:, :].rearrange("(r f) b -> f r b", f=F)
                    nc.sync.dma_start(
                        dfull[:, :].rearrange("f (r b) -> f r b", b=BL), dv)
                    nc.vector.tensor_add(xdT[0:F, :], xdT[0:F, :], dfull[:])

    nc.compile()
    return nc


_CACHE = {}


def kernel(struct_obs_ab, struct_obs_c, params):
    x_ab = _np(struct_obs_ab)            # (17, 256, 56)
    pose0 = _np(struct_obs_c)[0]         # (256, 56)

    wd = _prep_weights(params)
    wshapes = {k: v.shape for k, v in wd.items()}

    if "prog" not in _CACHE:
        _CACHE["prog"] = build_program(wshapes)
    nc = _CACHE["prog"]

    xfT = np.ascontiguousarray(x_ab.transpose(0, 2, 1))   # (17, 56, 256)
    pose0T = np.ascontiguousarray(pose0.T)                # (56, 256)

    in_maps = []
    for c in range(N_CORES):
        sl = slice(c * BL, (c + 1) * BL)
        # (57, 17*32): col 32t+b = [x_ab[t, local b]; 1]
        xla = np.concatenate(
            [x_ab[:, sl, :].transpose(0, 2, 1),
             np.ones((TAB, 1, BL), np.float32)], axis=1)   # (17, 57, 32)
        xla = np.ascontiguousarray(
            xla.transpose(1, 0, 2).reshape(F + 1, TAB * BL))
        m = dict(wd)
        m["xfT"] = xfT
        m["xlT_all"] = xla
        m["pose0T_full"] = pose0T
        m["pose0T_loc"] = np.ascontiguousarray(pose0T[:, sl])
        m["pose0_loc_b"] = np.ascontiguousarray(pose0[sl, :])
        in_maps.append(m)

    res = run_bass_kernel_spmd(nc, in_maps, core_ids=list(range(N_CORES)))
    _CACHE["last_results"] = res

    poses = np.zeros((1, TPRED, K, F), np.float32)
    stab = np.zeros((1, K), np.float32)
    for c in range(N_CORES):
        sl = slice(c * BL, (c + 1) * BL)
        poses[0, :, sl, :] = res.results[c]["poses_loc"]
        stab[0, sl] = res.results[c]["stab_loc"][0]

    stability = np.broadcast_to(stab[:, None, :], (1, TPRED, K)).copy()
    return poses, stability


# revision 22
# speedup vs baseline: 1.2147x; 1.0299x over previous
"""CoPhyNet Trainium2 kernel — 8-core SPMD Bass/Tile implementation.

Self-contained: hardcodes shapes from the problem spec.
  struct_obs_ab: (17, 256, 56) fp32
  struct_obs_c:  (1, 256, 56) fp32

Sharding: the object axis K=256 is split 8 ways (32 "local" objects per
core). All-pairs edge MLPs: pair[p,q] = concat(x[q], x[p]), output index q
(local), mean over p (free axis). Layer 1 is decomposed into an outer sum
U[q] + V[p]; V is streamed over all 256 p as the matmul moving operand,
U enters as the per-partition activation bias. Layers 2/3 run as 4-way
block-diagonal [128,128] @ [128,256] float32r matmuls (4 local objects
packed in the partition dim). The delta loop all-gathers each core's
32-row V contribution (4 KB) per iteration.
"""

import numpy as np

import concourse.bass as bass
import concourse.bacc as bacc
import concourse.tile as tile
import concourse.mybir as mybir
from concourse.bass_utils import run_bass_kernel_spmd

FP = mybir.dt.float32
FR = mybir.dt.float32r
AF = mybir.ActivationFunctionType
ALU = mybir.AluOpType

N_CORES = 8
TAB = 17
K = 256
F = 56
H = 32
BL = K // N_CORES          # local objects per core = 32
NB = BL // 4               # 4-packed blocks per core = 8
TPRED = TAB - 1
G = H + F                  # 88
DEBUG = False


def _np(x):
    return np.asarray(x, dtype=np.float32)


def _blockdiag4(w):
    out = np.zeros((128, 128), dtype=np.float32)
    for j in range(4):
        out[32 * j:32 * j + 32, 32 * j:32 * j + 32] = w
    return out


def _prep_weights(params):
    """Host-side weight preprocessing -> dict of np arrays (DRAM inputs)."""
    d = {}

    def lin(p):
        return _np(p["w"]), _np(p["b"])

    # ---- phase 1: mlp_inter (112->32->32->32) ----
    w1, b1 = lin(params["mlp_inter"][0])
    w2, b2 = lin(params["mlp_inter"][1])
    w3, b3 = lin(params["mlp_inter"][2])
    d["w1t_aug"] = np.concatenate([w1[:F], b1[None, :]], 0)          # (57, 32)
    d["w1b4"] = np.tile(w1[F:], (1, 4))                               # (56, 128)
    d["w2bd"] = _blockdiag4(w2)                                       # (128, 128)
    d["w3bd"] = _blockdiag4(w3)
    d["b2_4"] = np.tile(b2, 4)[:, None]                               # (128, 1)
    d["b3_4"] = np.tile(b3, 4)[:, None]

    # ---- mlp_out (88->32->32), E rows prescaled by 1/K ----
    wo1, bo1 = lin(params["mlp_out"][0])
    wo2, bo2 = lin(params["mlp_out"][1])
    d["wo1a"] = np.concatenate([wo1[:F], bo1[None, :]], 0)            # (57, 32)
    d["wo1b"] = wo1[F:] / K                                           # (32, 32)


    # ---- GRUs: split gate weights; x-side fused with the upstream linear
    # (gates = W_ih^T @ (Wup^T @ v) = (Wup @ W_ih)^T @ v, exact) ----
    _gru_raw = {}
    for name, p in [("r", params["rnn"]), ("rd", params["rnn_delta"])]:
        wih, whh = _np(p["w_ih"]), _np(p["w_hh"])
        bih, bhh = _np(p["b_ih"]), _np(p["b_hh"])
        _gru_raw[name] = wih
        d[f"whh_{name}_rz"] = whh[:, 0:2 * H]                         # (32, 64)
        d[f"whh_{name}_n"] = whh[:, 2 * H:]
        bs = bih + bhh
        d[f"bs_{name}_r"] = bs[0:H, None]                             # (32, 1)
        d[f"bs_{name}_z"] = bs[H:2 * H, None]
        d[f"bhhn_{name}"] = bhh[2 * H:, None]
        d[f"bihn_{name}"] = bih[2 * H:, None]

    # ---- phase 3: mlp_inter_stab (176->32->32->32), xc = [conf, pose] ----
    ws1, bs1 = lin(params["mlp_inter_stab"][0])
    ws2, bs2 = lin(params["mlp_inter_stab"][1])
    ws3, bs3 = lin(params["mlp_inter_stab"][2])
    d["w1st_aug"] = np.concatenate([ws1[:G], bs1[None, :]], 0)        # (89, 32)
    d["w1sb4"] = np.tile(ws1[G:], (1, 4))                             # (88, 128)
    d["w2sbd"] = _blockdiag4(ws2)
    d["w3sbd"] = _blockdiag4(ws3)
    d["b2s_4"] = np.tile(bs2, 4)[:, None]
    d["b3s_4"] = np.tile(bs3, 4)[:, None]

    # ---- mlp_stab (120->32->1), Es rows prescaled ----
    wm1, bm1 = lin(params["mlp_stab"][0])
    wm2, bm2 = lin(params["mlp_stab"][1])
    d["ws1a"] = np.concatenate([wm1[:G], bm1[None, :]], 0)            # (89, 32)
    d["ws1b"] = wm1[G:] / K                                           # (32, 32)
    d["ws2_aug"] = np.concatenate([wm2, bm2[None, :]], 0)             # (33, 1)

    # ---- phase 4: mlp_inter_delta (176->...), xcat = [pose, conf] ----
    wd1, bd1 = lin(params["mlp_inter_delta"][0])
    wd2, bd2 = lin(params["mlp_inter_delta"][1])
    wd3, bd3 = lin(params["mlp_inter_delta"][2])
    d["w1dt_aug"] = np.concatenate([wd1[:G], bd1[None, :]], 0)        # (89, 32)
    d["w1db4"] = np.tile(wd1[G:], (1, 4))                             # (88, 128)
    d["w2dbd"] = _blockdiag4(wd2)
    d["w3dbd"] = _blockdiag4(wd3)
    d["b2d_4"] = np.tile(bd2, 4)[:, None]
    d["b3d_4"] = np.tile(bd3, 4)[:, None]

    # ---- mlp_gcn_delta (120->32->32), Ed rows prescaled ----
    wg1, bg1 = lin(params["mlp_gcn_delta"][0])
    wg2, bg2 = lin(params["mlp_gcn_delta"][1])
    d["wg1a"] = np.concatenate([wg1[:G], bg1[None, :]], 0)            # (89, 32)
    d["wg1b"] = wg1[G:] / K                                           # (32, 32)
    for nm, wup in [("r", wo2), ("rd", wg2)]:
        wih = _gru_raw[nm]
        d[f"wx_{nm}_rz"] = wup @ wih[:, 0:2 * H]                      # (32, 64)
        d[f"wx_{nm}_n"] = wup @ wih[:, 2 * H:]                        # (32, 32)
    # fold the upstream linear's bias through the gate weights
    for nm, bias in [("r", bo2), ("rd", bg2)]:
        wih = _gru_raw[nm]
        d[f"bs_{nm}_rz"] = np.concatenate(
            [d.pop(f"bs_{nm}_r"), d.pop(f"bs_{nm}_z")], 0) \
            + (bias @ wih[:, 0:2 * H])[:, None]                       # (64, 1)
        d[f"bihn_{nm}"] = d[f"bihn_{nm}"] + (bias @ wih[:, 2 * H:])[:, None]

    # ---- fc_delta (32->56) ----
    wf, bf = lin(params["fc_delta"])
    d["wfc_aug"] = np.concatenate([wf, bf[None, :]], 0)               # (33, 56)

    d["ident56"] = np.eye(F, dtype=np.float32)                        # (56, 56)
    d["ones_1x56"] = np.ones((1, F), dtype=np.float32)                # (1, 56)
    d["ones_fr"] = np.ones((1, K), dtype=np.float32)                  # (1, 256)
    return d


# everything that feeds a matmul is float32r (single-pass PE); fp32 only for
# bias columns (activation bias / tensor_scalar operands)
_FP_WEIGHTS = {
    "b2_4", "b3_4", "b2s_4", "b3s_4", "b2d_4", "b3d_4",
    "bs_r_rz", "bhhn_r", "bihn_r",
    "bs_rd_rz", "bhhn_rd", "bihn_rd",
}


class _P:
    """Pools holder."""


def _interleave(nc, p, psU_ap, n_groups):
    """ub[32j+f, g] = U[f, 4g+j]; psU_ap [32, 4*n_groups] PSUM -> SBUF ub."""
    ub = p.wk.tile([128, n_groups], FP, tag="ub")
    sv = psU_ap.rearrange("f (g j) -> f g j", j=4)
    for j in range(4):
        if j % 2 == 0:
            nc.scalar.copy(ub[32 * j:32 * j + 32, :], sv[:, :, j])
        else:
            nc.vector.tensor_copy(ub[32 * j:32 * j + 32, :], sv[:, :, j])
    return ub


def _deinterleave(nc, dst_ap, src):
    """dst[f, 4g+j] = src[32j+f, g]; dst AP [32, 32] SBUF, src [128, 8]."""
    dv = dst_ap.rearrange("f (g j) -> f g j", j=4)
    for j in range(4):
        if j % 2 == 0:
            nc.scalar.copy(dv[:, :, j], src[32 * j:32 * j + 32, :])
        else:
            nc.vector.tensor_copy(dv[:, :, j], src[32 * j:32 * j + 32, :])


def _edge_blocks(nc, p, v4_ap, ub_cols, w2bd, w3bd, b2col, b3col, msum):
    """8 blocks of the 4-packed edge MLP, processed as 4 block-PAIRS with
    [128, 512] matmuls/passes; msum [128, 8] gets per-block sums.

    ub_cols(g) -> [128, 1] bias AP for block g.
    """
    for pr in range(NB // 2):
        g0, g1b = 2 * pr, 2 * pr + 1
        h1 = p.blk.tile([128, 512], FR, tag="h1")
        nc.scalar.activation(h1[:, 0:256], v4_ap, AF.Relu, bias=ub_cols(g0))
        nc.vector.scalar_tensor_tensor(h1[:, 256:512], v4_ap, ub_cols(g1b),
                                       p.zeros[:, 0:256],
                                       op0=ALU.add, op1=ALU.max)
        ps2 = p.ps2.tile([128, 512], FP, tag="mm")
        nc.tensor.matmul(ps2[:], w2bd[:], h1[:], start=True, stop=True)
        h2 = p.blk.tile([128, 512], FR, tag="h2")
        if pr % 2 == 0:
            nc.vector.tensor_scalar(h2[:], ps2[:], b2col, 0.0,
                                    op0=ALU.add, op1=ALU.max)
        else:
            nc.scalar.activation(h2[:], ps2[:], AF.Relu, bias=b2col)
        ps3 = p.ps3.tile([128, 512], FP, tag="mm")
        nc.tensor.matmul(ps3[:], w3bd[:], h2[:], start=True, stop=True)
        e3 = p.blk.tile([128, 512], FP, tag="e3")
        if pr % 2 == 0:
            # DVE: relu pass + strided free-axis reduce
            nc.vector.scalar_tensor_tensor(e3[:], ps3[:], b3col, p.zeros[:],
                                           op0=ALU.add, op1=ALU.max)
            ev = e3[:, :].rearrange("q (pair a) -> q pair a", pair=2)
            nc.vector.tensor_reduce(msum[:, g0:g1b + 1], ev,
                                    op=ALU.add, axis=mybir.AxisListType.X)
        else:
            # ACT: two relu halves with fused accumulators
            nc.scalar.activation(e3[:, 0:256], ps3[:, 0:256], AF.Relu,
                                 bias=b3col, accum_out=msum[:, g0:g0 + 1])
            nc.scalar.activation(e3[:, 256:512], ps3[:, 256:512], AF.Relu,
                                 bias=b3col, accum_out=msum[:, g1b:g1b + 1])


def _gru_step(nc, p, W, pre, x_ap, h_ap):
    """One feature-major GRU cell step; h_ap [32, BL] updated in place.

    x_ap is the pre-GRU relu vector (with ones row); the upstream linear is
    folded into the wx_* gate weights.
    """
    ps_rz = p.pssm.tile([2 * H, BL], FP, tag="sm")
    nc.tensor.matmul(ps_rz[:], W[f"wx_{pre}_rz"][:], x_ap,
                     start=True, stop=False)
    nc.tensor.matmul(ps_rz[:], W[f"whh_{pre}_rz"][:], h_ap,
                     start=False, stop=True)
    rz = p.wk.tile([2 * H, BL], FP, tag="rz")
    nc.scalar.activation(rz[:], ps_rz[:], AF.Sigmoid,
                         bias=W[f"bs_{pre}_rz"][:])
    z = p.wk.tile([H, BL], FP, tag="z")
    nc.vector.tensor_copy(z[:], rz[H:2 * H, :])
    ps_gin = p.pssm.tile([H, BL], FP, tag="sm")
    nc.tensor.matmul(ps_gin[:], W[f"wx_{pre}_n"][:], x_ap,
                     start=True, stop=True)
    ps_ghn = p.pssm.tile([H, BL], FP, tag="sm")
    nc.tensor.matmul(ps_ghn[:], W[f"whh_{pre}_n"][:], h_ap,
                     start=True, stop=True)
    hn = p.wk.tile([H, BL], FP, tag="hn")
    nc.scalar.activation(hn[:], ps_ghn[:], AF.Identity,
                         bias=W[f"bhhn_{pre}"][:])
    rhn = p.wk.tile([H, BL], FP, tag="rhn")
    nc.vector.tensor_mul(rhn[:], rz[0:H, :], hn[:])
    npre = p.wk.tile([H, BL], FP, tag="npre")
    nc.vector.tensor_add(npre[:], ps_gin[:], rhn[:])
    nt = p.wk.tile([H, BL], FP, tag="nt")
    nc.scalar.activation(nt[:], npre[:], AF.Tanh, bias=W[f"bihn_{pre}"][:])
    hmn = p.wk.tile([H, BL], FP, tag="hmn")
    nc.vector.tensor_sub(hmn[:], h_ap, nt[:])
    zh = p.wk.tile([H, BL], FP, tag="zh")
    nc.vector.tensor_mul(zh[:], z[:], hmn[:])
    nc.vector.tensor_add(h_ap, nt[:], zh[:])


def build_program(wshapes):
    """Build + compile the 8-core SPMD program. wshapes: weight name->shape."""
    nc = bacc.Bacc("TRN2", target_bir_lowering=False, debug=False,
                   num_devices=N_CORES)

    # ---------- DRAM I/O ----------
    Wd = {}
    for name, shp in wshapes.items():
        dt = FP if name in _FP_WEIGHTS else FR
        Wd[name] = nc.dram_tensor(name, list(shp), dt,
                                  kind="ExternalInput").ap()

    xfT_d = nc.dram_tensor("xfT", [TAB, F, K], FR, kind="ExternalInput").ap()
    # all 17 t's of local x, feature-major with ones row: (57, 544)
    xla_d = nc.dram_tensor("xlT_all", [F + 1, TAB * BL], FR,
                           kind="ExternalInput").ap()
    pose0T_full_d = nc.dram_tensor("pose0T_full", [F, K], FR,
                                   kind="ExternalInput").ap()
    pose0T_loc_d = nc.dram_tensor("pose0T_loc", [F, BL], FR,
                                  kind="ExternalInput").ap()
    pose0_loc_b_d = nc.dram_tensor("pose0_loc_b", [BL, F], FP,
                                   kind="ExternalInput").ap()

    poses_out = nc.dram_tensor("poses_loc", [TPRED, BL, F], FP,
                               kind="ExternalOutput").ap()
    stab_out = nc.dram_tensor("stab_loc", [1, BL], FP,
                              kind="ExternalOutput").ap()
    dbg = {}
    if DEBUG:
        for nm, shp in [("dbg_conf", [H, BL]), ("dbg_em0", [H, BL]),
                        ("dbg_es", [H, BL]), ("dbg_ed0", [H, BL])]:
            dbg[nm] = nc.dram_tensor(nm, shp, FR, kind="ExternalOutput").ap()
        dbg["dbg_mask"] = nc.dram_tensor("dbg_mask", [F, BL], FP,
                                         kind="ExternalOutput").ap()

    rg = [list(range(N_CORES))]

    with tile.TileContext(nc) as tc:
        with (
            tc.tile_pool(name="const", bufs=1) as cpool,
            tc.tile_pool(name="state", bufs=1) as st,
            tc.tile_pool(name="xin", bufs=4) as xin,
            tc.tile_pool(name="work", bufs=8) as wk,
            tc.tile_pool(name="blk", bufs=6) as blk,
            tc.tile_pool(name="psV", bufs=2, space="PSUM") as psV_pool,
            tc.tile_pool(name="psmm", bufs=4, space="PSUM") as psmm_pool,
            tc.tile_pool(name="pssm", bufs=2, space="PSUM") as pssm,
            tc.tile_pool(name="dram", bufs=4, space="DRAM") as dram,
        ):
            p = _P()
            p.wk, p.blk, p.pssm = wk, blk, pssm
            p.ps2, p.ps3 = psmm_pool, psmm_pool
            p.zeros = cpool.tile([128, 512], FP, tag="zeros")
            nc.vector.memset(p.zeros[:], 0.0)

            # ---- load constants into SBUF ----
            W = {}
            for name, shp in wshapes.items():
                dt = FP if name in _FP_WEIGHTS else FR
                t = cpool.tile(list(shp), dt, tag=f"c_{name}")
                nc.sync.dma_start(t[:], Wd[name][:])
                W[name] = t

            # ---- persistent state ----
            hconf = st.tile([H, BL], FR, tag="hconf")     # phase-2 GRU state
            nc.vector.tensor_copy(hconf[:], p.zeros[0:H, 0:BL])
            # ginT = xcat^T local: rows 0:56 pose, 56:88 conf, 88 ones
            ginT = st.tile([G + 1, BL], FR, tag="ginT")
            nc.sync.dma_start(ginT[0:F, :], pose0T_loc_d[:])
            nc.sync.dma_start(ginT[G:G + 1, :], Wd["ones_fr"][:, 0:BL])
            # xcT_full: rows 0:32 conf^T full, 32:88 pose0^T full, 88 ones
            xcT = st.tile([G + 1, K], FR, tag="xcT")
            nc.sync.dma_start(xcT[H:G, :], pose0T_full_d[:])
            nc.sync.dma_start(xcT[G:G + 1, :], Wd["ones_fr"][:])
            # xcl = xc^T local: rows 0:32 conf, 32:88 pose, 88 ones
            xcl = st.tile([G + 1, BL], FR, tag="xcl")
            nc.sync.dma_start(xcl[H:G, :], pose0T_loc_d[:])
            nc.sync.dma_start(xcl[G:G + 1, :], Wd["ones_fr"][:, 0:BL])
            # hd_aug: GRU-delta state + ones row
            hd_aug = st.tile([H + 1, BL], FR, tag="hd_aug")
            nc.vector.tensor_copy(hd_aug[0:H, :], p.zeros[0:H, 0:BL])
            nc.sync.dma_start(hd_aug[H:H + 1, :], Wd["ones_fr"][:, 0:BL])
            pose_b = st.tile([BL, F], FP, tag="pose_b")   # b-major pose copy
            nc.sync.dma_start(pose_b[:], pose0_loc_b_d[:])
            mask56 = st.tile([F, BL], FR, tag="mask56")
            # persistent relu tile with ones row (stab head)
            s1 = st.tile([H + 1, BL], FR, tag="s1")
            nc.sync.dma_start(s1[H:H + 1, :], Wd["ones_fr"][:, 0:BL])
            # full xcat^T = [pose; conf] over all 256 objects (phase-4 V side)
            xdT = st.tile([G, K], FR, tag="xdT")
            nc.sync.dma_start(xdT[0:F, :], pose0T_full_d[:])
            # whole local x batch (feature-major + ones rows)
            xla = st.tile([F + 1, TAB * BL], FR, tag="xla")
            nc.sync.dma_start(xla[:], xla_d[:])

            # ============ batched U for phase 1: ub_all [128, 136] ==========
            # U cols are (t, b): col = 32t + b, b = 4g + j; ub col = 8t + g
            ub_all = st.tile([128, TAB * NB], FP, tag="ub_all")
            for c0, c1 in [(0, 256), (256, TAB * BL)]:
                psUh = pssm.tile([H, c1 - c0], FP, tag="sm")
                nc.tensor.matmul(psUh[:], W["w1t_aug"][:], xla[:, c0:c1],
                                 start=True, stop=True)
                sv = psUh[:, :].rearrange("f (g j) -> f g j", j=4)
                gc0 = c0 // 4
                ng = (c1 - c0) // 4
                for j in range(4):
                    if j % 2 == 0:
                        nc.scalar.copy(
                            ub_all[32 * j:32 * j + 32, gc0:gc0 + ng],
                            sv[:, :, j])
                    else:
                        nc.vector.tensor_copy(
                            ub_all[32 * j:32 * j + 32, gc0:gc0 + ng],
                            sv[:, :, j])

            # ================= phase 1 + 2: gcn_on_AB + GRU =================
            for t in range(TAB):
                xf = xin.tile([F, K], FR, tag="xf")
                nc.sync.dma_start(xf[:], xfT_d[t])

                psV = psV_pool.tile([128, K], FP, tag="psV")
                nc.tensor.matmul(psV[:], W["w1b4"][:], xf[:],
                                 start=True, stop=True)
                msum = wk.tile([128, NB], FP, tag="msum")
                _edge_blocks(nc, p, psV[:],
                             lambda g, t=t: ub_all[:, 8 * t + g:8 * t + g + 1],
                             W["w2bd"], W["w3bd"],
                             W["b2_4"][:], W["b3_4"][:], msum)

                emT = wk.tile([H, BL], FR, tag="emT")
                _deinterleave(nc, emT[:, :], msum)

                pso1 = pssm.tile([H, BL], FP, tag="sm")
                nc.tensor.matmul(pso1[:], W["wo1a"][:],
                                 xla[:, t * BL:(t + 1) * BL],
                                 start=True, stop=False)
                nc.tensor.matmul(pso1[:], W["wo1b"][:], emT[:],
                                 start=False, stop=True)
                q1 = wk.tile([H, BL], FR, tag="q1")
                nc.scalar.activation(q1[:], pso1[:], AF.Relu)
                if DEBUG and t == 0:
                    nc.sync.dma_start(dbg["dbg_em0"][:], emT[:])

                _gru_step(nc, p, W, "r", q1[:], hconf[:])

            # conf into ginT/xcl (SBUF->SBUF DMA handles row offsets)
            nc.sync.dma_start(ginT[F:G, :], hconf[:])
            nc.sync.dma_start(xcl[0:H, :], hconf[:])
            if DEBUG:
                nc.sync.dma_start(dbg["dbg_conf"][:], hconf[:])

            # ================= conf AllGather =================
            cin = dram.tile([H, BL], FR, tag="cin")
            nc.sync.dma_start(cin[:], hconf[:])
            cout = dram.tile([K, BL], FR, tag="cout")
            nc.gpsimd.collective_compute(
                "AllGather", ALU.bypass, replica_groups=rg,
                ins=[cin.opt()], outs=[cout.opt()])
            cview = cout[:, :].rearrange("(r f) b -> f r b", f=H)
            nc.sync.dma_start(
                xcT[0:H, :].rearrange("f (r b) -> f r b", b=BL), cview)
            nc.sync.dma_start(
                xdT[F:G, :].rearrange("f (r b) -> f r b", b=BL), cview)

            # ================= phase 3: pred_stab =================
            psUs = pssm.tile([H, BL], FP, tag="sm")
            nc.tensor.matmul(psUs[:], W["w1st_aug"][:], xcl[:],
                             start=True, stop=True)
            ubs = _interleave(nc, p, psUs[:, :], NB)
            psVs = psV_pool.tile([128, K], FP, tag="psV")
            nc.tensor.matmul(psVs[:], W["w1sb4"][:], xcT[0:G, :],
                             start=True, stop=True)
            msums = wk.tile([128, NB], FP, tag="msum")
            _edge_blocks(nc, p, psVs[:], lambda g: ubs[:, g:g + 1],
                         W["w2sbd"], W["w3sbd"],
                         W["b2s_4"][:], W["b3s_4"][:], msums)
            esT = wk.tile([H, BL], FR, tag="esT")
            _deinterleave(nc, esT[:, :], msums)
            if DEBUG:
                nc.sync.dma_start(dbg["dbg_es"][:], esT[:])

            pss1 = pssm.tile([H, BL], FP, tag="sm")
            nc.tensor.matmul(pss1[:], W["ws1a"][:], xcl[:],
                             start=True, stop=False)
            nc.tensor.matmul(pss1[:], W["ws1b"][:], esT[:],
                             start=False, stop=True)
            nc.scalar.activation(s1[0:H, :], pss1[:], AF.Relu)
            pss2 = pssm.tile([1, BL], FP, tag="sm")
            nc.tensor.matmul(pss2[:], W["ws2_aug"][:], s1[:],
                             start=True, stop=True)
            stab_sb = wk.tile([1, BL], FP, tag="stab_sb")
            nc.scalar.copy(stab_sb[:], pss2[:])
            nc.sync.dma_start(stab_out[:], stab_sb[:])
            # mask row: 1.0 where stab <= 0
            maskr = wk.tile([1, BL], FR, tag="maskr")
            nc.vector.tensor_scalar(maskr[:], pss2[:], 0.0, None,
                                    op0=ALU.is_le)
            psm = pssm.tile([F, BL], FP, tag="sm")
            nc.tensor.matmul(psm[:], W["ones_1x56"][:], maskr[:],
                             start=True, stop=True)
            nc.vector.tensor_copy(mask56[:], psm[:])
            if DEBUG:
                mask56fp = wk.tile([F, BL], FP, tag="mask56fp")
                nc.vector.tensor_copy(mask56fp[:], psm[:])
                nc.sync.dma_start(dbg["dbg_mask"][:], mask56fp[:])

            # ================= phase 4: delta loop =================
            dfull_prev = None
            for i in range(TPRED):
                psV4 = psV_pool.tile([128, K], FP, tag="psV")
                # V from last iteration's xcat (available before the AG)...
                nc.tensor.matmul(psV4[:], W["w1db4"][:], xdT[:],
                                 start=True, stop=(dfull_prev is None))
                if dfull_prev is not None:
                    # ...plus the just-gathered delta contribution
                    nc.tensor.matmul(psV4[:], W["w1db4"][0:F, :],
                                     dfull_prev[:], start=False, stop=True)
                    # fold the delta into xcat for the next iteration
                    nc.vector.tensor_add(xdT[0:F, :], xdT[0:F, :],
                                         dfull_prev[:])
                psUd = pssm.tile([H, BL], FP, tag="sm")
                nc.tensor.matmul(psUd[:], W["w1dt_aug"][:], ginT[:],
                                 start=True, stop=True)
                ubd = _interleave(nc, p, psUd[:, :], NB)
                msumd = wk.tile([128, NB], FP, tag="msum")
                _edge_blocks(nc, p, psV4[:], lambda g: ubd[:, g:g + 1],
                             W["w2dbd"], W["w3dbd"],
                             W["b2d_4"][:], W["b3d_4"][:], msumd)
                edT = wk.tile([H, BL], FR, tag="edT")
                _deinterleave(nc, edT[:, :], msumd)
                if DEBUG and i == 0:
                    nc.sync.dma_start(dbg["dbg_ed0"][:], edT[:])

                psg1 = pssm.tile([H, BL], FP, tag="sm")
                nc.tensor.matmul(psg1[:], W["wg1a"][:], ginT[:],
                                 start=True, stop=False)
                nc.tensor.matmul(psg1[:], W["wg1b"][:], edT[:],
                                 start=False, stop=True)
                g1 = wk.tile([H, BL], FR, tag="g1")
                nc.scalar.activation(g1[:], psg1[:], AF.Relu)

                _gru_step(nc, p, W, "rd", g1[:], hd_aug[0:H, :])

                psd = pssm.tile([F, BL], FP, tag="sm")
                nc.tensor.matmul(psd[:], W["wfc_aug"][:], hd_aug[:],
                                 start=True, stop=True)
                delta = wk.tile([F, BL], FR, tag="delta")
                nc.vector.tensor_mul(delta[:], psd[:], mask56[:])
                # pose update (feature-major, in place)
                nc.vector.tensor_add(ginT[0:F, :], ginT[0:F, :], delta[:])

                # b-major pose snapshot -> DRAM output
                psdT = pssm.tile([BL, F], FR, tag="sm")
                nc.tensor.transpose(psdT[:], delta[:], W["ident56"][:])
                nc.vector.tensor_add(pose_b[:], pose_b[:], psdT[:])
                nc.sync.dma_start(poses_out[i], pose_b[:])

                if i < TPRED - 1:
                    # all-gather this iteration's delta; update full xcat
                    din = dram.tile([F, BL], FR, tag="din")
                    nc.sync.dma_start(din[:], delta[:])
                    dout = dram.tile([N_CORES * F, BL], FR, tag="dout")
                    nc.gpsimd.collective_compute(
                        "AllGather", ALU.bypass, replica_groups=rg,
                        ins=[din.opt()], outs=[dout.opt()])
                    dfull = wk.tile([F, K], FR, tag="dfull")
                    dv = dout[:, :].rearrange("(r f) b -> f r b", f=F)
                    nc.sync.dma_start(
                        dfull[:, :].rearrange("f (r b) -> f r b", b=BL), dv)
                    dfull_prev = dfull

    nc.compile()
    return nc


_CACHE = {}


def kernel(struct_obs_ab, struct_obs_c, params):
    x_ab = _np(struct_obs_ab)            # (17, 256, 56)
    pose0 = _np(struct_obs_c)[0]         # (256, 56)

    wd = _prep_weights(params)
    wshapes = {k: v.shape for k, v in wd.items()}

    if "prog" not in _CACHE:
        _CACHE["prog"] = build_program(wshapes)
    nc = _CACHE["prog"]

    xfT = np.ascontiguousarray(x_ab.transpose(0, 2, 1))   # (17, 56, 256)
    pose0T = np.ascontiguousarray(pose0.T)                # (56, 256)

    in_maps = []
    for c in range(N_CORES):
        sl = slice(c * BL, (c + 1) * BL)
        # (57, 17*32): col 32t+b = [x_ab[t, local b]; 1]
        xla = np.concatenate(
            [x_ab[:, sl, :].transpose(0, 2, 1),
             np.ones((TAB, 1, BL), np.float32)], axis=1)   # (17, 57, 32)
        xla = np.ascontiguousarray(
            xla.transpose(1, 0, 2).reshape(F + 1, TAB * BL))
        m = dict(wd)
        m["xfT"] = xfT
        m["xlT_all"] = xla
        m["pose0T_full"] = pose0T
        m["pose0T_loc"] = np.ascontiguousarray(pose0T[:, sl])
        m["pose0_loc_b"] = np.ascontiguousarray(pose0[sl, :])
        in_maps.append(m)

    res = run_bass_kernel_spmd(nc, in_maps, core_ids=list(range(N_CORES)))
    _CACHE["last_results"] = res

    poses = np.zeros((1, TPRED, K, F), np.float32)
    stab = np.zeros((1, K), np.float32)
    for c in range(N_CORES):
        sl = slice(c * BL, (c + 1) * BL)
        poses[0, :, sl, :] = res.results[c]["poses_loc"]
        stab[0, sl] = res.results[c]["stab_loc"][0]

    stability = np.broadcast_to(stab[:, None, :], (1, TPRED, K)).copy()
    return poses, stability


# revision 32
# speedup vs baseline: 1.2357x; 1.0172x over previous
"""CoPhyNet Trainium2 kernel — 8-core SPMD Bass/Tile implementation.

Self-contained: hardcodes shapes from the problem spec.
  struct_obs_ab: (17, 256, 56) fp32
  struct_obs_c:  (1, 256, 56) fp32

Sharding: the object axis K=256 is split 8 ways (32 "local" objects per
core). All-pairs edge MLPs: pair[p,q] = concat(x[q], x[p]), output index q
(local), mean over p (free axis). Layer 1 is decomposed into an outer sum
U[q] + V[p]; V is streamed over all 256 p as the matmul moving operand,
U enters as the per-partition activation bias. Layers 2/3 run as 4-way
block-diagonal [128,128] @ [128,256] float32r matmuls (4 local objects
packed in the partition dim). The delta loop all-gathers each core's
32-row V contribution (4 KB) per iteration.
"""

import ml_dtypes
import numpy as np

import concourse.bass as bass
import concourse.bacc as bacc
import concourse.tile as tile
import concourse.mybir as mybir
from concourse.bass_utils import run_bass_kernel_spmd

FP = mybir.dt.float32
FR = mybir.dt.float32r
BF = mybir.dt.bfloat16
AF = mybir.ActivationFunctionType
ALU = mybir.AluOpType

N_CORES = 8
TAB = 17
K = 256
F = 56
H = 32
BL = K // N_CORES          # local objects per core = 32
NB = BL // 4               # 4-packed blocks per core = 8
TPRED = TAB - 1
G = H + F                  # 88
DEBUG = False


def _np(x):
    return np.asarray(x, dtype=np.float32)


def _blockdiag4(w):
    out = np.zeros((128, 128), dtype=np.float32)
    for j in range(4):
        out[32 * j:32 * j + 32, 32 * j:32 * j + 32] = w
    return out


def _prep_weights(params):
    """Host-side weight preprocessing -> dict of np arrays (DRAM inputs)."""
    d = {}

    def lin(p):
        return _np(p["w"]), _np(p["b"])

    # ---- phase 1: mlp_inter (112->32->32->32) ----
    w1, b1 = lin(params["mlp_inter"][0])
    w2, b2 = lin(params["mlp_inter"][1])
    w3, b3 = lin(params["mlp_inter"][2])
    d["w1t_aug"] = np.concatenate([w1[:F], b1[None, :]], 0)          # (57, 32)
    d["w1b4"] = np.tile(w1[F:], (1, 4))                               # (56, 128)
    d["w2bd"] = _blockdiag4(w2)                                       # (128, 128)
    d["w3bd"] = _blockdiag4(w3)
    d["b2_4"] = np.tile(b2, 4)[:, None]                               # (128, 1)
    d["b3_4"] = np.tile(b3, 4)[:, None]

    # ---- mlp_out (88->32->32), E rows prescaled by 1/K ----
    wo1, bo1 = lin(params["mlp_out"][0])
    wo2, bo2 = lin(params["mlp_out"][1])
    d["wo1a"] = np.concatenate([wo1[:F], bo1[None, :]], 0)            # (57, 32)
    d["wo1b"] = wo1[F:] / K                                           # (32, 32)


    # ---- GRUs: split gate weights; x-side fused with the upstream linear
    # (gates = W_ih^T @ (Wup^T @ v) = (Wup @ W_ih)^T @ v, exact) ----
    _gru_raw = {}
    for name, p in [("r", params["rnn"]), ("rd", params["rnn_delta"])]:
        wih, whh = _np(p["w_ih"]), _np(p["w_hh"])
        bih, bhh = _np(p["b_ih"]), _np(p["b_hh"])
        _gru_raw[name] = wih
        d[f"whh_{name}_rz"] = whh[:, 0:2 * H]                         # (32, 64)
        d[f"whh_{name}_n"] = whh[:, 2 * H:]
        bs = bih + bhh
        d[f"bs_{name}_r"] = bs[0:H, None]                             # (32, 1)
        d[f"bs_{name}_z"] = bs[H:2 * H, None]
        d[f"bhhn_{name}"] = bhh[2 * H:, None]
        d[f"bihn_{name}"] = bih[2 * H:, None]

    # ---- phase 3: mlp_inter_stab (176->32->32->32), xc = [conf, pose] ----
    ws1, bs1 = lin(params["mlp_inter_stab"][0])
    ws2, bs2 = lin(params["mlp_inter_stab"][1])
    ws3, bs3 = lin(params["mlp_inter_stab"][2])
    d["w1st_aug"] = np.concatenate([ws1[:G], bs1[None, :]], 0)        # (89, 32)
    d["w1sb4"] = np.tile(ws1[G:], (1, 4))                             # (88, 128)
    d["w2sbd"] = _blockdiag4(ws2)
    d["w3sbd"] = _blockdiag4(ws3)
    d["b2s_4"] = np.tile(bs2, 4)[:, None]
    d["b3s_4"] = np.tile(bs3, 4)[:, None]

    # ---- mlp_stab (120->32->1), Es rows prescaled ----
    wm1, bm1 = lin(params["mlp_stab"][0])
    wm2, bm2 = lin(params["mlp_stab"][1])
    d["ws1a"] = np.concatenate([wm1[:G], bm1[None, :]], 0)            # (89, 32)
    d["ws1b"] = wm1[G:] / K                                           # (32, 32)
    d["ws2_aug"] = np.concatenate([wm2, bm2[None, :]], 0)             # (33, 1)

    # ---- phase 4: mlp_inter_delta (176->...), xcat = [pose, conf] ----
    wd1, bd1 = lin(params["mlp_inter_delta"][0])
    wd2, bd2 = lin(params["mlp_inter_delta"][1])
    wd3, bd3 = lin(params["mlp_inter_delta"][2])
    d["w1dt_aug"] = np.concatenate([wd1[:G], bd1[None, :]], 0)        # (89, 32)
    # xdT layout is [conf; pose], so reorder the V-side rows to match
    w1db_ro = np.concatenate([wd1[G + F:], wd1[G:G + F]], 0)          # (88, 32)
    d["w1db4"] = np.tile(w1db_ro, (1, 4)).astype(ml_dtypes.bfloat16)  # (88, 128)
    w1db4p = np.zeros((G, 128), dtype=np.float32)                     # (88, 128)
    w1db4p[H:G] = np.tile(wd1[G:G + F], (1, 4))
    d["w1db4p"] = w1db4p.astype(ml_dtypes.bfloat16)
    d["w2dbd"] = _blockdiag4(wd2)
    d["w3dbd"] = _blockdiag4(wd3)
    d["b2d_4"] = np.tile(bd2, 4)[:, None]
    d["b3d_4"] = np.tile(bd3, 4)[:, None]

    # ---- mlp_gcn_delta (120->32->32), Ed rows prescaled ----
    wg1, bg1 = lin(params["mlp_gcn_delta"][0])
    wg2, bg2 = lin(params["mlp_gcn_delta"][1])
    d["wg1a"] = np.concatenate([wg1[:G], bg1[None, :]], 0)            # (89, 32)
    d["wg1b"] = wg1[G:] / K                                           # (32, 32)
    for nm, wup in [("r", wo2), ("rd", wg2)]:
        wih = _gru_raw[nm]
        d[f"wx_{nm}_rz"] = wup @ wih[:, 0:2 * H]                      # (32, 64)
        d[f"wx_{nm}_n"] = wup @ wih[:, 2 * H:]                        # (32, 32)
    # fold the upstream linear's bias through the gate weights
    for nm, bias in [("r", bo2), ("rd", bg2)]:
        wih = _gru_raw[nm]
        d[f"bs_{nm}_rz"] = np.concatenate(
            [d.pop(f"bs_{nm}_r"), d.pop(f"bs_{nm}_z")], 0) \
            + (bias @ wih[:, 0:2 * H])[:, None]                       # (64, 1)
        d[f"bihn_{nm}"] = d[f"bihn_{nm}"] + (bias @ wih[:, 2 * H:])[:, None]

    # ---- fc_delta (32->56) ----
    wf, bf = lin(params["fc_delta"])
    d["wfc_aug"] = np.concatenate([wf, bf[None, :]], 0)               # (33, 56)

    d["ident56"] = np.eye(F, dtype=np.float32)                        # (56, 56)
    d["ones_1x56"] = np.ones((1, F), dtype=np.float32)                # (1, 56)
    d["ones_fr"] = np.ones((1, K), dtype=np.float32)                  # (1, 256)
    return d


# everything that feeds a matmul is float32r (single-pass PE); fp32 only for
# bias columns (activation bias / tensor_scalar operands)
_FP_WEIGHTS = {
    "b2_4", "b3_4", "b2s_4", "b3s_4", "b2d_4", "b3d_4",
    "bs_r_rz", "bhhn_r", "bihn_r",
    "bs_rd_rz", "bhhn_rd", "bihn_rd",
    "ident56",
}
_BF_WEIGHTS = {"w1db4", "w1db4p"}


def _wdtype(name):
    if name in _BF_WEIGHTS:
        return BF
    return FP if name in _FP_WEIGHTS else FR


class _P:
    """Pools holder."""


def _interleave(nc, p, psU_ap, n_groups):
    """ub[32j+f, g] = U[f, 4g+j]; psU_ap [32, 4*n_groups] PSUM -> SBUF ub."""
    ub = p.wk.tile([128, n_groups], FP, tag="ub")
    sv = psU_ap.rearrange("f (g j) -> f g j", j=4)
    for j in range(4):
        if j % 2 == 0:
            nc.scalar.copy(ub[32 * j:32 * j + 32, :], sv[:, :, j])
        else:
            nc.vector.tensor_copy(ub[32 * j:32 * j + 32, :], sv[:, :, j])
    return ub


def _deinterleave(nc, dst_ap, src):
    """dst[f, 4g+j] = src[32j+f, g]; dst AP [32, 32] SBUF, src [128, 8]."""
    dv = dst_ap.rearrange("f (g j) -> f g j", j=4)
    for j in range(4):
        if j % 2 == 0:
            nc.scalar.copy(dv[:, :, j], src[32 * j:32 * j + 32, :])
        else:
            nc.vector.tensor_copy(dv[:, :, j], src[32 * j:32 * j + 32, :])


def _edge_blocks(nc, p, v4_ap, ub_cols, w2bd, w3bd, b2col, b3col, msum,
                 hdt=FR):
    """8 blocks of the 4-packed edge MLP, processed as 4 block-PAIRS with
    [128, 512] matmuls/passes; msum [128, 8] gets per-block sums.

    ub_cols(g) -> [128, 1] bias AP for block g.
    """
    for pr in range(NB // 2):
        g0, g1b = 2 * pr, 2 * pr + 1
        h1 = p.blk.tile([128, 512], hdt, tag="h1")
        nc.scalar.activation(h1[:, 0:256], v4_ap, AF.Relu, bias=ub_cols(g0))
        nc.vector.scalar_tensor_tensor(h1[:, 256:512], v4_ap, ub_cols(g1b),
                                       p.zeros[:, 0:256],
                                       op0=ALU.add, op1=ALU.max)
        ps2 = p.ps2.tile([128, 512], FP, tag="mm")
        nc.tensor.matmul(ps2[:], w2bd[:], h1[:], start=True, stop=True)
        h2 = p.blk.tile([128, 512], hdt, tag="h2")
        if pr % 2 == 0:
            nc.vector.tensor_scalar(h2[:], ps2[:], b2col, 0.0,
                                    op0=ALU.add, op1=ALU.max)
        else:
            nc.scalar.activation(h2[:], ps2[:], AF.Relu, bias=b2col)
        ps3 = p.ps3.tile([128, 512], FP, tag="mm")
        nc.tensor.matmul(ps3[:], w3bd[:], h2[:], start=True, stop=True)
        e3 = p.blk.tile([128, 512], FP, tag="e3")
        if pr % 2 == 0:
            # DVE: relu pass + strided free-axis reduce
            nc.vector.scalar_tensor_tensor(e3[:], ps3[:], b3col, p.zeros[:],
                                           op0=ALU.add, op1=ALU.max)
            ev = e3[:, :].rearrange("q (pair a) -> q pair a", pair=2)
            nc.vector.tensor_reduce(msum[:, g0:g1b + 1], ev,
                                    op=ALU.add, axis=mybir.AxisListType.X)
        else:
            # ACT: two relu halves with fused accumulators
            nc.scalar.activation(e3[:, 0:256], ps3[:, 0:256], AF.Relu,
                                 bias=b3col, accum_out=msum[:, g0:g0 + 1])
            nc.scalar.activation(e3[:, 256:512], ps3[:, 256:512], AF.Relu,
                                 bias=b3col, accum_out=msum[:, g1b:g1b + 1])


def _gru_step(nc, p, W, pre, x_ap, h_ap):
    """One feature-major GRU cell step; h_ap [32, BL] updated in place.

    x_ap is the pre-GRU relu vector (with ones row); the upstream linear is
    folded into the wx_* gate weights.
    """
    ps_rz = p.pssm.tile([2 * H, BL], FP, tag="sm")
    nc.tensor.matmul(ps_rz[:], W[f"wx_{pre}_rz"][:], x_ap,
                     start=True, stop=False)
    nc.tensor.matmul(ps_rz[:], W[f"whh_{pre}_rz"][:], h_ap,
                     start=False, stop=True)
    rz = p.wk.tile([2 * H, BL], FP, tag="rz")
    nc.scalar.activation(rz[:], ps_rz[:], AF.Sigmoid,
                         bias=W[f"bs_{pre}_rz"][:])
    z = p.wk.tile([H, BL], FP, tag="z")
    nc.vector.tensor_copy(z[:], rz[H:2 * H, :])
    ps_gin = p.pssm.tile([H, BL], FP, tag="sm")
    nc.tensor.matmul(ps_gin[:], W[f"wx_{pre}_n"][:], x_ap,
                     start=True, stop=True)
    ps_ghn = p.pssm.tile([H, BL], FP, tag="sm")
    nc.tensor.matmul(ps_ghn[:], W[f"whh_{pre}_n"][:], h_ap,
                     start=True, stop=True)
    hn = p.wk.tile([H, BL], FP, tag="hn")
    nc.scalar.activation(hn[:], ps_ghn[:], AF.Identity,
                         bias=W[f"bhhn_{pre}"][:])
    rhn = p.wk.tile([H, BL], FP, tag="rhn")
    nc.vector.tensor_mul(rhn[:], rz[0:H, :], hn[:])
    npre = p.wk.tile([H, BL], FP, tag="npre")
    nc.vector.tensor_add(npre[:], ps_gin[:], rhn[:])
    nt = p.wk.tile([H, BL], FP, tag="nt")
    nc.scalar.activation(nt[:], npre[:], AF.Tanh, bias=W[f"bihn_{pre}"][:])
    hmn = p.wk.tile([H, BL], FP, tag="hmn")
    nc.vector.tensor_sub(hmn[:], h_ap, nt[:])
    zh = p.wk.tile([H, BL], FP, tag="zh")
    nc.vector.tensor_mul(zh[:], z[:], hmn[:])
    nc.vector.tensor_add(h_ap, nt[:], zh[:])


def build_program(wshapes):
    """Build + compile the 8-core SPMD program. wshapes: weight name->shape."""
    nc = bacc.Bacc("TRN2", target_bir_lowering=False, debug=False,
                   num_devices=N_CORES)

    # ---------- DRAM I/O ----------
    Wd = {}
    for name, shp in wshapes.items():
        Wd[name] = nc.dram_tensor(name, list(shp), _wdtype(name),
                                  kind="ExternalInput").ap()

    xfT_d = nc.dram_tensor("xfT", [TAB, F, K], FR, kind="ExternalInput").ap()
    # all 17 t's of local x, feature-major with ones row: (57, 544)
    xla_d = nc.dram_tensor("xlT_all", [F + 1, TAB * BL], FR,
                           kind="ExternalInput").ap()
    pose0T_full_d = nc.dram_tensor("pose0T_full", [F, K], FR,
                                   kind="ExternalInput").ap()
    pose0T_full_bf_d = nc.dram_tensor("pose0T_full_bf", [F, K], BF,
                                      kind="ExternalInput").ap()
    pose0T_loc_d = nc.dram_tensor("pose0T_loc", [F, BL], FR,
                                  kind="ExternalInput").ap()
    pose0_loc_b_d = nc.dram_tensor("pose0_loc_b", [BL, F], FP,
                                   kind="ExternalInput").ap()

    poses_out = nc.dram_tensor("poses_loc", [TPRED, BL, F], FP,
                               kind="ExternalOutput").ap()
    stab_out = nc.dram_tensor("stab_loc", [1, BL], FP,
                              kind="ExternalOutput").ap()
    dbg = {}
    if DEBUG:
        for nm, shp in [("dbg_conf", [H, BL]), ("dbg_em0", [H, BL]),
                        ("dbg_es", [H, BL]), ("dbg_ed0", [H, BL]),
                        ("dbg_v40", [H, K])]:
            dbg[nm] = nc.dram_tensor(nm, shp, FR, kind="ExternalOutput").ap()
        dbg["dbg_mask"] = nc.dram_tensor("dbg_mask", [F, BL], FP,
                                         kind="ExternalOutput").ap()
        dbg["dbg_ub0"] = nc.dram_tensor("dbg_ub0", [128, NB], FP,
                                        kind="ExternalOutput").ap()

    rg = [list(range(N_CORES))]

    with tile.TileContext(nc) as tc:
        with (
            tc.tile_pool(name="const", bufs=1) as cpool,
            tc.tile_pool(name="state", bufs=1) as st,
            tc.tile_pool(name="xin", bufs=4) as xin,
            tc.tile_pool(name="work", bufs=8) as wk,
            tc.tile_pool(name="blk", bufs=6) as blk,
            tc.tile_pool(name="psV", bufs=2, space="PSUM") as psV_pool,
            tc.tile_pool(name="psmm", bufs=4, space="PSUM") as psmm_pool,
            tc.tile_pool(name="pssm", bufs=2, space="PSUM") as pssm,
            tc.tile_pool(name="dram", bufs=4, space="DRAM") as dram,
        ):
            p = _P()
            p.wk, p.blk, p.pssm = wk, blk, pssm
            p.ps2, p.ps3 = psmm_pool, psmm_pool
            p.zeros = cpool.tile([128, 512], FP, tag="zeros")
            nc.vector.memset(p.zeros[:], 0.0)

            # ---- load constants into SBUF ----
            W = {}
            for name, shp in wshapes.items():
                t = cpool.tile(list(shp), _wdtype(name), tag=f"c_{name}")
                nc.sync.dma_start(t[:], Wd[name][:])
                W[name] = t

            # ---- persistent state ----
            hconf = st.tile([H, BL], FR, tag="hconf")     # phase-2 GRU state
            nc.vector.tensor_copy(hconf[:], p.zeros[0:H, 0:BL])
            # ginT = xcat^T local: rows 0:56 pose, 56:88 conf, 88 ones
            ginT = st.tile([G + 1, BL], FR, tag="ginT")
            nc.sync.dma_start(ginT[0:F, :], pose0T_loc_d[:])
            nc.sync.dma_start(ginT[G:G + 1, :], Wd["ones_fr"][:, 0:BL])
            # xcT_full: rows 0:32 conf^T full, 32:88 pose0^T full, 88 ones
            xcT = st.tile([G + 1, K], FR, tag="xcT")
            nc.sync.dma_start(xcT[H:G, :], pose0T_full_d[:])
            nc.sync.dma_start(xcT[G:G + 1, :], Wd["ones_fr"][:])
            # xcl = xc^T local: rows 0:32 conf, 32:88 pose, 88 ones
            xcl = st.tile([G + 1, BL], FR, tag="xcl")
            nc.sync.dma_start(xcl[H:G, :], pose0T_loc_d[:])
            nc.sync.dma_start(xcl[G:G + 1, :], Wd["ones_fr"][:, 0:BL])
            # hd_aug: GRU-delta state + ones row
            hd_aug = st.tile([H + 1, BL], FR, tag="hd_aug")
            nc.vector.tensor_copy(hd_aug[0:H, :], p.zeros[0:H, 0:BL])
            nc.sync.dma_start(hd_aug[H:H + 1, :], Wd["ones_fr"][:, 0:BL])
            pose_b = st.tile([BL, F], FP, tag="pose_b")   # b-major pose copy
            nc.sync.dma_start(pose_b[:], pose0_loc_b_d[:])
            mask56 = st.tile([F, BL], FR, tag="mask56")
            # persistent relu tile with ones row (stab head)
            s1 = st.tile([H + 1, BL], FR, tag="s1")
            nc.sync.dma_start(s1[H:H + 1, :], Wd["ones_fr"][:, 0:BL])
            # full xcat^T, rows 0:32 conf / 32:88 pose (phase-4 V side)
            xdT = st.tile([G, K], BF, tag="xdT")
            nc.sync.dma_start(xdT[H:G, :], pose0T_full_bf_d[:])
            # whole local x batch (feature-major + ones rows)
            xla = st.tile([F + 1, TAB * BL], FR, tag="xla")
            nc.sync.dma_start(xla[:], xla_d[:])

            # ============ batched U for phase 1: ub_all [128, 136] ==========
            # U cols are (t, b): col = 32t + b, b = 4g + j; ub col = 8t + g
            ub_all = st.tile([128, TAB * NB], FP, tag="ub_all")
            for c0, c1 in [(0, 256), (256, TAB * BL)]:
                psUh = pssm.tile([H, c1 - c0], FP, tag="sm")
                nc.tensor.matmul(psUh[:], W["w1t_aug"][:], xla[:, c0:c1],
                                 start=True, stop=True)
                sv = psUh[:, :].rearrange("f (g j) -> f g j", j=4)
                gc0 = c0 // 4
                ng = (c1 - c0) // 4
                for j in range(4):
                    if j % 2 == 0:
                        nc.scalar.copy(
                            ub_all[32 * j:32 * j + 32, gc0:gc0 + ng],
                            sv[:, :, j])
                    else:
                        nc.vector.tensor_copy(
                            ub_all[32 * j:32 * j + 32, gc0:gc0 + ng],
                            sv[:, :, j])

            # ================= phase 1 + 2: gcn_on_AB + GRU =================
            for t in range(TAB):
                xf = xin.tile([F, K], FR, tag="xf")
                nc.sync.dma_start(xf[:], xfT_d[t])

                psV = psV_pool.tile([128, K], FP, tag="psV")
                nc.tensor.matmul(psV[:], W["w1b4"][:], xf[:],
                                 start=True, stop=True)
                msum = wk.tile([128, NB], FP, tag="msum")
                _edge_blocks(nc, p, psV[:],
                             lambda g, t=t: ub_all[:, 8 * t + g:8 * t + g + 1],
                             W["w2bd"], W["w3bd"],
                             W["b2_4"][:], W["b3_4"][:], msum)

                emT = wk.tile([H, BL], FR, tag="emT")
                _deinterleave(nc, emT[:, :], msum)

                pso1 = pssm.tile([H, BL], FP, tag="sm")
                nc.tensor.matmul(pso1[:], W["wo1a"][:],
                                 xla[:, t * BL:(t + 1) * BL],
                                 start=True, stop=False)
                nc.tensor.matmul(pso1[:], W["wo1b"][:], emT[:],
                                 start=False, stop=True)
                q1 = wk.tile([H, BL], FR, tag="q1")
                nc.scalar.activation(q1[:], pso1[:], AF.Relu)
                if DEBUG and t == 0:
                    nc.sync.dma_start(dbg["dbg_em0"][:], emT[:])

                _gru_step(nc, p, W, "r", q1[:], hconf[:])

            # conf into ginT/xcl (SBUF->SBUF DMA handles row offsets)
            nc.sync.dma_start(ginT[F:G, :], hconf[:])
            nc.sync.dma_start(xcl[0:H, :], hconf[:])
            if DEBUG:
                nc.sync.dma_start(dbg["dbg_conf"][:], hconf[:])

            # ================= conf AllGather =================
            cin = dram.tile([H, BL], FR, tag="cin")
            nc.sync.dma_start(cin[:], hconf[:])
            cout = dram.tile([K, BL], FR, tag="cout")
            nc.gpsimd.collective_compute(
                "AllGather", ALU.bypass, replica_groups=rg,
                ins=[cin.opt()], outs=[cout.opt()])
            cview = cout[:, :].rearrange("(r f) b -> f r b", f=H)
            nc.sync.dma_start(
                xcT[0:H, :].rearrange("f (r b) -> f r b", b=BL), cview)
            conf_fp = wk.tile([H, K], FP, tag="conf_fp")
            nc.vector.tensor_copy(conf_fp[:], xcT[0:H, :])
            nc.vector.tensor_copy(xdT[0:H, :], conf_fp[:])

            # ================= phase 3: pred_stab =================
            psUs = pssm.tile([H, BL], FP, tag="sm")
            nc.tensor.matmul(psUs[:], W["w1st_aug"][:], xcl[:],
                             start=True, stop=True)
            ubs = _interleave(nc, p, psUs[:, :], NB)
            psVs = psV_pool.tile([128, K], FP, tag="psV")
            nc.tensor.matmul(psVs[:], W["w1sb4"][:], xcT[0:G, :],
                             start=True, stop=True)
            msums = wk.tile([128, NB], FP, tag="msum")
            _edge_blocks(nc, p, psVs[:], lambda g: ubs[:, g:g + 1],
                         W["w2sbd"], W["w3sbd"],
                         W["b2s_4"][:], W["b3s_4"][:], msums)
            esT = wk.tile([H, BL], FR, tag="esT")
            _deinterleave(nc, esT[:, :], msums)
            if DEBUG:
                nc.sync.dma_start(dbg["dbg_es"][:], esT[:])

            pss1 = pssm.tile([H, BL], FP, tag="sm")
            nc.tensor.matmul(pss1[:], W["ws1a"][:], xcl[:],
                             start=True, stop=False)
            nc.tensor.matmul(pss1[:], W["ws1b"][:], esT[:],
                             start=False, stop=True)
            nc.scalar.activation(s1[0:H, :], pss1[:], AF.Relu)
            pss2 = pssm.tile([1, BL], FP, tag="sm")
            nc.tensor.matmul(pss2[:], W["ws2_aug"][:], s1[:],
                             start=True, stop=True)
            stab_sb = wk.tile([1, BL], FP, tag="stab_sb")
            nc.scalar.copy(stab_sb[:], pss2[:])
            nc.sync.dma_start(stab_out[:], stab_sb[:])
            # mask row: 1.0 where stab <= 0
            maskr = wk.tile([1, BL], FR, tag="maskr")
            nc.vector.tensor_scalar(maskr[:], pss2[:], 0.0, None,
                                    op0=ALU.is_le)
            psm = pssm.tile([F, BL], FP, tag="sm")
            nc.tensor.matmul(psm[:], W["ones_1x56"][:], maskr[:],
                             start=True, stop=True)
            nc.vector.tensor_copy(mask56[:], psm[:])
            if DEBUG:
                mask56fp = wk.tile([F, BL], FP, tag="mask56fp")
                nc.vector.tensor_copy(mask56fp[:], psm[:])
                nc.sync.dma_start(dbg["dbg_mask"][:], mask56fp[:])

            # ================= phase 4: delta loop =================
            dfull_prev = None
            for i in range(TPRED):
                psV4 = psV_pool.tile([128, K], FP, tag="psV")
                # V from last iteration's xcat (available before the AG)...
                nc.tensor.matmul(psV4[:], W["w1db4"][:], xdT[:],
                                 start=True, stop=(dfull_prev is None))
                if dfull_prev is not None:
                    # ...plus the just-gathered delta contribution
                    nc.tensor.matmul(psV4[:], W["w1db4p"][:],
                                     dfull_prev[:],
                                     start=False, stop=True)
                    # fold the delta into xcat for the next iteration
                    nc.vector.tensor_add(xdT[H:2 * H, :], xdT[H:2 * H, :],
                                         dfull_prev[H:2 * H, :])
                    nc.vector.tensor_add(xdT[2 * H:G, :], xdT[2 * H:G, :],
                                         dfull_prev[2 * H:G, :])
                psUd = pssm.tile([H, BL], FP, tag="sm")
                nc.tensor.matmul(psUd[:], W["w1dt_aug"][:], ginT[:],
                                 start=True, stop=True)
                ubd = _interleave(nc, p, psUd[:, :], NB)
                msumd = wk.tile([128, NB], FP, tag="msum")
                _edge_blocks(nc, p, psV4[:], lambda g: ubd[:, g:g + 1],
                             W["w2dbd"], W["w3dbd"],
                             W["b2d_4"][:], W["b3d_4"][:], msumd)
                edT = wk.tile([H, BL], FR, tag="edT")
                _deinterleave(nc, edT[:, :], msumd)
                if DEBUG and i == 0:
                    nc.sync.dma_start(dbg["dbg_ed0"][:], edT[:])
                    v4fp = wk.tile([H, K], FR, tag="v4fp")
                    nc.vector.tensor_copy(v4fp[:], psV4[0:H, :])
                    nc.sync.dma_start(dbg["dbg_v40"][:], v4fp[:])
                    nc.sync.dma_start(dbg["dbg_ub0"][:], ubd[:])

                psg1 = pssm.tile([H, BL], FP, tag="sm")
                nc.tensor.matmul(psg1[:], W["wg1a"][:], ginT[:],
                                 start=True, stop=False)
                nc.tensor.matmul(psg1[:], W["wg1b"][:], edT[:],
                                 start=False, stop=True)
                g1 = wk.tile([H, BL], FR, tag="g1")
                nc.scalar.activation(g1[:], psg1[:], AF.Relu)

                _gru_step(nc, p, W, "rd", g1[:], hd_aug[0:H, :])

                psd = pssm.tile([F, BL], FP, tag="sm")
                nc.tensor.matmul(psd[:], W["wfc_aug"][:], hd_aug[:],
                                 start=True, stop=True)
                delta = wk.tile([F, BL], FP, tag="delta")
                nc.vector.tensor_mul(delta[:], psd[:], mask56[:])
                # pose update (feature-major, in place)
                nc.vector.tensor_add(ginT[0:F, :], ginT[0:F, :], delta[:])

                # b-major pose snapshot -> DRAM output
                psdT = pssm.tile([BL, F], FP, tag="sm")
                nc.tensor.transpose(psdT[:], delta[:], W["ident56"][:])
                nc.vector.tensor_add(pose_b[:], pose_b[:], psdT[:])
                nc.sync.dma_start(poses_out[i], pose_b[:])

                if i < TPRED - 1:
                    # all-gather this iteration's delta; update full xcat
                    dbf = wk.tile([F, BL], BF, tag="dbf")
                    nc.vector.tensor_copy(dbf[:], delta[:])
                    din = dram.tile([F, BL], BF, tag="din")
                    nc.sync.dma_start(din[:], dbf[:])
                    dout = dram.tile([N_CORES * F, BL], BF, tag="dout")
                    nc.gpsimd.collective_compute(
                        "AllGather", ALU.bypass, replica_groups=rg,
                        ins=[din.opt()], outs=[dout.opt()])
                    dfull = wk.tile([G, K], BF, tag="dfull")
                    nc.vector.memset(dfull[0:H, :], 0.0)
                    dv = dout[:, :].rearrange("(r f) b -> f r b", f=F)
                    nc.sync.dma_start(
                        dfull[H:G, :].rearrange("f (r b) -> f r b", b=BL), dv)
                    dfull_prev = dfull

    nc.compile()
    return nc


_CACHE = {}


def kernel(struct_obs_ab, struct_obs_c, params):
    x_ab = _np(struct_obs_ab)            # (17, 256, 56)
    pose0 = _np(struct_obs_c)[0]         # (256, 56)

    wd = _prep_weights(params)
    wshapes = {k: v.shape for k, v in wd.items()}

    if "prog" not in _CACHE:
        _CACHE["prog"] = build_program(wshapes)
    nc = _CACHE["prog"]

    xfT = np.ascontiguousarray(x_ab.transpose(0, 2, 1))   # (17, 56, 256)
    pose0T = np.ascontiguousarray(pose0.T)                # (56, 256)

    in_maps = []
    for c in range(N_CORES):
        sl = slice(c * BL, (c + 1) * BL)
        # (57, 17*32): col 32t+b = [x_ab[t, local b]; 1]
        xla = np.concatenate(
            [x_ab[:, sl, :].transpose(0, 2, 1),
             np.ones((TAB, 1, BL), np.float32)], axis=1)   # (17, 57, 32)
        xla = np.ascontiguousarray(
            xla.transpose(1, 0, 2).reshape(F + 1, TAB * BL))
        m = dict(wd)
        m["xfT"] = xfT
        m["xlT_all"] = xla
        m["pose0T_full"] = pose0T
        m["pose0T_full_bf"] = pose0T.astype(ml_dtypes.bfloat16)
        m["pose0T_loc"] = np.ascontiguousarray(pose0T[:, sl])
        m["pose0_loc_b"] = np.ascontiguousarray(pose0[sl, :])
        in_maps.append(m)

    res = run_bass_kernel_spmd(nc, in_maps, core_ids=list(range(N_CORES)))
    _CACHE["last_results"] = res

    poses = np.zeros((1, TPRED, K, F), np.float32)
    stab = np.zeros((1, K), np.float32)
    for c in range(N_CORES):
        sl = slice(c * BL, (c + 1) * BL)
        poses[0, :, sl, :] = res.results[c]["poses_loc"]
        stab[0, sl] = res.results[c]["stab_loc"][0]

    stability = np.broadcast_to(stab[:, None, :], (1, TPRED, K)).copy()
    return poses, stability
